# revision 1
# baseline (speedup 1.0000x reference)
"""Trainium2 Bass kernel for nn_LHFA_76278619177511.

Fused transposed-attention block (LHFA):
  q = dwconv3(conv1x1(x, Wq), Wq_dw)   (same for k from y, v from z)
  attn = softmax(l2norm(q) @ l2norm(k)^T * temp)   per-head [32,32]
  out = Wproj @ (attn @ v) + Wfus @ [x;y;z]

Strategy (per core, pure data-parallel over batch B=8 on 8 cores):
  - The depthwise 3x3 is folded into the 1x1 conv: merged weight
    W9[c,(tap,k)] = dw[c,tap]*W1[c,k], contracted over K=576 in 5
    PE K-steps using zero-padded input copies (A = padded image,
    B = A shifted +1 col, D = A shifted +1 row) so every shifted tap
    is a plain rectangular AP read.
  - q,k are produced directly TRANSPOSED ([n,c], data-stationary
    matmuls) so the per-head Gram matrix q@k^T needs no transposes;
    v is produced in natural layout [c,n] for attn@v.
  - Row norms via ones-matmul over squared qT/kT; softmax math on the
    tiny [256,32] per-head blocks with DVE 32x32 block transposes.
  - Everything bf16 in SBUF, fp32 accumulation in PSUM.
"""

import numpy as np
import ml_dtypes

import bass_rust
import concourse.bass as bass
import concourse.mybir as mybir
from concourse import tile as tile_mod
from concourse.tile import TileContext
from concourse.vector_clock import ScopedClock
from concourse.bass_utils import run_bass_kernel_spmd

BF16 = mybir.dt.bfloat16
F32 = mybir.dt.float32

C = 64          # input channels
DIM = 256       # q/k/v channels
HEADS = 8
H = W = 128
N = H * W       # 16384
PW = W + 2      # padded row length 130
HB = 16         # band height (output rows per band)
NB = H // HB    # 8 bands
TW = (HB + 2) * PW  # 2340 cols per pad tile

# 5 K-step scheme: (src_tile, K, ap_offset, v_row_off, v_col_off)
#   src 0 = AB pair tile (A rows 0:64, B = A<<1col rows 64:128)
#   src 1 = AD pair tile (A rows 0:64, D = A<<1row rows 64:128)
STEPS = [
    (0, 128, 0 * PW + 0),
    (0, 128, 1 * PW + 0),
    (0, 128, 2 * PW + 0),
    (1, 128, 0 * PW + 2),
    (1, 64, 2 * PW + 2),
]
# taps (dy,dx) per step/half for weight merging
STEP_TAPS = [
    [(-1, -1), (-1, 0)],
    [(0, -1), (0, 0)],
    [(1, -1), (1, 0)],
    [(-1, 1), (0, 1)],
    [(1, 1)],
]

_PATCHED = False


def _patch_tile_drain():
    """This walrus build rejects >1 sem wait on a CTRL (Drain) instruction;
    split the TileContext tail-drain waits onto individual nops."""
    global _PATCHED
    if _PATCHED:
        return
    _PATCHED = True

    def _drain_and_barrier(self, tick_clock, wait_clock):
        nc = self.nc
        drain_inst = nc.sync.drain()
        wait_clock.add_sem_waits(
            drain_inst.ins, ScopedClock({None: tick_clock.global_clock})
        )
        si = drain_inst.ins.sync_info
        waits = list(si.on_wait or [])
        if len(waits) > 1:
            si.on_wait = waits[:1]
            for w in waits[1:]:
                nop = nc.sync.nop(nofuse=True, hint="split_wait")
                nop.ins.sync_info = bass_rust.SyncInfo(on_wait=[w], on_update=[])
        nc.all_engine_barrier()
        assert self.sems is not None
        popped = nc._tile_sem_poison_stack.pop()
        assert popped is self._sem_poison
        nc.clear_and_free_semaphores(list(self.sems.allocated().values()))
        nc.all_engine_barrier()

    tile_mod.TileContext._drain_and_barrier = _drain_and_barrier
    try:
        from concourse import tile_utils
        tile_utils.max_sbuf_usage = 208 * 1024
    except Exception:
        pass



def _split_excess_waits(nc, max_waits=1):
    """This walrus build caps sem waits per instruction encoding; hoist
    excess waits onto preceding same-engine NoOps (queues are in-order,
    so a wait satisfied on an earlier instruction orders the later one)."""
    import bass_rust as _br

    ctr = [0]
    for f in nc.m.functions:
        for blk in f.blocks:
            out = []
            for inst in blk.instructions:
                si = inst.sync_info
                waits = list(si.on_wait) if (si and si.on_wait) else []
                if len(waits) > max_waits:
                    keep = waits[:max_waits]
                    extra = waits[max_waits:]
                    si.on_wait = keep
                    for w in extra:
                        ctr[0] += 1
                        nop = _br.InstNoOp(name=f"wsplit-{ctr[0]}", ins=[], outs=[])
                        nop.engine = inst.engine
                        nop.sync_info = _br.SyncInfo(on_wait=[w], on_update=[])
                        try:
                            nc.register_instruction(nop, overwrite=True)
                        except Exception:
                            pass
                        out.append(nop)
                out.append(inst)
            blk.instructions[:] = out


def _merge_w(W1, Wdw):
    """-> [128, 5, 256] float32: [p, s, c] = dw[c, tap(s, p//64)] * W1[c, p%64]."""
    out = np.zeros((128, 5, 256), np.float32)
    W1 = W1[:, :, 0, 0]  # [256, 64]
    for s, taps in enumerate(STEP_TAPS):
        for half, (dy, dx) in enumerate(taps):
            out[half * 64 : (half + 1) * 64, s, :] = (
                Wdw[:, 0, 1 + dy, 1 + dx][:, None] * W1
            ).T
    return out


def _bf(a):
    return np.ascontiguousarray(a).astype(ml_dtypes.bfloat16)


def _build_nc(wq, wk, wv, wprojT, wfusT, temp_cols):
    """Build the Bass module. Weight arrays are merged/transposed fp32."""
    _patch_tile_drain()
    nc = bass.Bass()

    xd = nc.declare_dram_parameter("x", [C, N], BF16, isOutput=False)
    yd = nc.declare_dram_parameter("y", [C, N], BF16, isOutput=False)
    zd = nc.declare_dram_parameter("z", [C, N], BF16, isOutput=False)
    od = nc.declare_dram_parameter("out", [DIM, N], F32, isOutput=True)

    wq_d = nc.inline_tensor(_bf(wq.reshape(128, 5 * 256)), name="wq9")
    wk_d = nc.inline_tensor(_bf(wk.reshape(128, 5 * 256)), name="wk9")
    wv_d = nc.inline_tensor(_bf(wv.reshape(128, 5 * 256)), name="wv9")
    wp_d = nc.inline_tensor(_bf(wprojT), name="wprojT")  # [128, 512]
    wf_d = nc.inline_tensor(_bf(wfusT), name="wfusT")    # [128, 512]
    tc0_d = nc.inline_tensor(np.ascontiguousarray(temp_cols[0]), name="tcol0")
    id_d = nc.inline_tensor(np.eye(128, dtype=ml_dtypes.bfloat16), name="ident")
    tc1_d = nc.inline_tensor(np.ascontiguousarray(temp_cols[1]), name="tcol1")

    with TileContext(nc) as tc:
        import contextlib

        with contextlib.ExitStack() as ctx:
            wpool = ctx.enter_context(tc.tile_pool(name="wpool", bufs=1))
            vpool = ctx.enter_context(tc.tile_pool(name="vpool", bufs=1))
            pads = ctx.enter_context(tc.tile_pool(name="pads", bufs=2))
            qkp = ctx.enter_context(tc.tile_pool(name="qkp", bufs=4))
            smallp = ctx.enter_context(tc.tile_pool(name="smallp", bufs=2))
            p2p = ctx.enter_context(tc.tile_pool(name="p2p", bufs=3))

            # --- weights to SBUF ---
            wq_sb = wpool.tile([128, 5 * 256], BF16, tag="wq")
            wk_sb = wpool.tile([128, 5 * 256], BF16, tag="wk")
            wv_sb = wpool.tile([128, 5 * 256], BF16, tag="wv")
            wp_sb = wpool.tile([128, 512], BF16, tag="wp")
            wf_sb = wpool.tile([128, 512], BF16, tag="wf")
            ident_sb = wpool.tile([128, 128], BF16, tag="ident")
            tcol = [wpool.tile([128, 1], F32, tag=f"tc{i}", name=f"tcol{i}") for i in range(2)]
            nc.scalar.dma_start(out=wq_sb, in_=wq_d[:])
            nc.scalar.dma_start(out=wk_sb, in_=wk_d[:])
            nc.scalar.dma_start(out=wv_sb, in_=wv_d[:])

            # --- persistent state ---
            v_slab = [vpool.tile([128, N], BF16, tag=f"v{mb}", name=f"vslab{mb}") for mb in range(2)]
            p1stack = ctx.enter_context(contextlib.ExitStack())
            ps_qk = p1stack.enter_context(tc.tile_pool(name="ps_qk", bufs=3, space="PSUM"))
            ps_v = p1stack.enter_context(tc.tile_pool(name="ps_v", bufs=3, space="PSUM"))
            ps_acc = p1stack.enter_context(tc.tile_pool(name="ps_acc", bufs=1, space="PSUM"))
            acc1 = ps_acc.tile([128, 512], F32, tag="acc1")
            acc2 = ps_acc.tile([128, 256], F32, tag="acc2")
            par_all = acc1[:, 0:256]
            pgq = acc1[:, 256:512]
            pgk = acc2

            ins_d = [xd, yd, zd]

            # ================= pass 1: bands =================
            for b in range(NB):
                lr0 = 1 if b == 0 else 0
                nr = (HB + 2) - (1 if b == 0 else 0) - (1 if b == NB - 1 else 0)
                ir0 = max(0, HB * b - 1)

                srcs = []  # per input: (AB, AD)
                tile_engs = [
                    (nc.sync, nc.gpsimd),
                    (nc.scalar, nc.sync),
                    (nc.gpsimd, nc.scalar),
                ]
                for ti, td in enumerate(ins_d):
                    nm = "xyz"[ti]
                    eAB, eAD = tile_engs[ti]
                    AB = pads.tile([128, TW], BF16, tag=f"{nm}AB")
                    AD = pads.tile([128, TW], BF16, tag=f"{nm}AD")
                    src_img = td[:].rearrange("p (r c) -> p r c", c=W)[
                        :, ir0 : ir0 + nr, :
                    ]
                    for T, de in ((AB, eAB), (AD, eAD)):
                        view = T[0:64, :].rearrange("p (r c) -> p r c", c=PW)
                        nc.vector.memset(view[:, :, 0:1], 0.0)
                        nc.gpsimd.memset(view[:, :, 129:130], 0.0)
                        if b == 0:
                            nc.gpsimd.memset(view[:, 0:1, :], 0.0)
                        if b == NB - 1:
                            nc.gpsimd.memset(view[:, HB + 1 : HB + 2, :], 0.0)
                        de.dma_start(
                            out=view[:, lr0 : lr0 + nr, 1 : 1 + W], in_=src_img
                        )
                    eAB.dma_start(out=AB[64:128, 0 : TW - 1], in_=AB[0:64, 1:TW])
                    eAD.dma_start(
                        out=AD[64:128, 0 : TW - PW], in_=AD[0:64, PW:TW]
                    )
                    srcs.append((AB, AD))

                # qT/kT convs + attnraw + sumsq per output row
                for hl in range(HB):
                    g = HB * b + hl
                    first, last = g == 0, g == H - 1
                    base = hl * PW
                    pqk_t = ps_qk.tile([128, 512], F32, tag="pqk")
                    pk_t = pqk_t[:, 0:256]
                    pq_t = pqk_t[:, 256:512]
                    for which, (w_sb, p_t) in enumerate(
                        ((wq_sb, pq_t), (wk_sb, pk_t))
                    ):
                        AB, AD = srcs[which]
                        for s, (st, K, off) in enumerate(STEPS):
                            src = (AB, AD)[st]
                            nc.tensor.matmul(
                                p_t,
                                lhsT=src[0:K, bass.ds(base + off, 128)],
                                rhs=w_sb[0:K, bass.ds(s * 256, 256)],
                                start=(s == 0),
                                stop=(s == 4),
                            )
                    cat = qkp.tile([128, 512], BF16, tag="cat")
                    if hl % 2 == 0:
                        nc.scalar.copy(cat, pqk_t)
                    else:
                        nc.vector.tensor_copy(cat, pqk_t)
                    for mb in range(2):
                        qsl = bass.ds(256 + mb * 128, 128)
                        ksl = bass.ds(mb * 128, 128)
                        nc.tensor.matmul(
                            par_all[:, bass.ds(mb * 128, 128)],
                            lhsT=cat[:, qsl],
                            rhs=cat[:, ksl],
                            start=first,
                            stop=last,
                            skip_group_check=True,
                        )
                        nc.tensor.matmul(
                            pgq[:, bass.ds(mb * 128, 128)],
                            lhsT=cat[:, qsl],
                            rhs=cat[:, qsl],
                            start=first,
                            stop=last,
                            skip_group_check=True,
                        )
                        nc.tensor.matmul(
                            pgk[:, bass.ds(mb * 128, 128)],
                            lhsT=cat[:, ksl],
                            rhs=cat[:, ksl],
                            start=first,
                            stop=last,
                            skip_group_check=True,
                        )

                # v conv (natural layout), 4 chunks of 4 rows
                zAB, zAD = srcs[2]
                zviews = [
                    T[:, :].rearrange("p (r c) -> p r c", c=PW) for T in (zAB, zAD)
                ]
                for cc in range(HB // 4):
                    hl0 = 4 * cc
                    for mb in range(2):
                        pv_t = ps_v.tile([128, 512], F32, tag="pv")
                        for s, (st, K, off) in enumerate(STEPS):
                            rs, cs = divmod(off, PW)
                            rhs = zviews[st][0:K, hl0 + rs : hl0 + rs + 4, cs : cs + 128]
                            nc.tensor.matmul(
                                pv_t,
                                lhsT=wv_sb[0:K, bass.ds(s * 256 + mb * 128, 128)],
                                rhs=rhs,
                                start=(s == 0),
                                stop=(s == 4),
                            )
                        dst = v_slab[mb][:, bass.ds((HB * b + hl0) * W, 512)]
                        if mb == 0:
                            nc.scalar.copy(dst, pv_t)
                        else:
                            nc.vector.tensor_copy(dst, pv_t)

            nc.sync.dma_start(out=wp_sb, in_=wp_d[:])
            nc.sync.dma_start(out=wf_sb, in_=wf_d[:])
            nc.sync.dma_start(out=tcol[0], in_=tc0_d[:])
            nc.sync.dma_start(out=tcol[1], in_=tc1_d[:])
            nc.sync.dma_start(out=ident_sb, in_=id_d[:])

            # ================= phase 1.5: softmax on [256, 32] =================
            ar_sb = [smallp.tile([128, 128], F32, tag=f"arsb{mb}", name=f"arsb{mb}") for mb in range(2)]
            nc.scalar.copy(ar_sb[0], par_all[:, 0:128])
            nc.scalar.copy(ar_sb[1], par_all[:, 128:256])
            bd = [smallp.tile([128, 128], BF16, tag=f"bd{mb}", name=f"bdiag{mb}") for mb in range(2)]
            for mb in range(2):
                scr = smallp.tile([128, 128], F32, tag="scr")
                rnq_c = smallp.tile([128, 1], F32, tag="rnq")
                rnk_c = smallp.tile([128, 1], F32, tag="rnk")
                for g_ps, dst in ((pgq, rnq_c), (pgk, rnk_c)):
                    ssum = smallp.tile([128, 1], F32, tag="ssum")
                    nc.vector.tensor_mul(scr, g_ps[:, bass.ds(mb * 128, 128)], ident_sb)
                    nc.vector.reduce_sum(out=ssum, in_=scr, axis=mybir.AxisListType.X)
                    nc.scalar.sqrt(ssum, ssum)
                    nc.vector.tensor_scalar_max(ssum, ssum, 1e-12)
                    nc.vector.reciprocal(dst, ssum)
                rnqt = smallp.tile([128, 1], F32, tag="rnqt")
                nc.vector.tensor_mul(rnqt, rnq_c, tcol[mb])

                hd = smallp.tile([128, 32], F32, tag="hd")
                for i in range(4):
                    nc.vector.tensor_copy(
                        hd[32 * i : 32 * (i + 1), :],
                        ar_sb[mb][32 * i : 32 * (i + 1), bass.ds(32 * i, 32)],
                    )
                hds = smallp.tile([128, 32], F32, tag="hds")
                nc.scalar.activation(
                    hds, hd, mybir.ActivationFunctionType.Copy, bias=0.0, scale=rnqt
                )
                hdT = smallp.tile([128, 32], F32, tag="hdT")
                nc.vector.transpose(hdT, hds)
                hdTs = smallp.tile([128, 32], F32, tag="hdTs")
                nc.scalar.activation(
                    hdTs, hdT, mybir.ActivationFunctionType.Copy, bias=0.0, scale=rnk_c
                )
                hd3 = smallp.tile([128, 32], F32, tag="hd3")
                nc.vector.transpose(hd3, hdTs)
                nmx = smallp.tile([128, 1], F32, tag="nmx")
                nc.vector.reduce_max(
                    out=nmx, in_=hd3, axis=mybir.AxisListType.X, negate=True
                )
                ex = smallp.tile([128, 32], F32, tag="ex")
                nc.scalar.activation(
                    ex, hd3, mybir.ActivationFunctionType.Exp, bias=nmx, scale=1.0
                )
                sm = smallp.tile([128, 1], F32, tag="sm")
                nc.vector.reduce_sum(out=sm, in_=ex, axis=mybir.AxisListType.X)
                rsm = smallp.tile([128, 1], F32, tag="rsm")
                nc.vector.reciprocal(rsm, sm)
                Pt = smallp.tile([128, 32], F32, tag="Pt")
                nc.scalar.activation(
                    Pt, ex, mybir.ActivationFunctionType.Copy, bias=0.0, scale=rsm
                )
                PtT = smallp.tile([128, 32], F32, tag="PtT")
                nc.vector.transpose(PtT, Pt)
                nc.gpsimd.memset(bd[mb], 0.0)
                for i in range(4):
                    nc.vector.tensor_copy(
                        bd[mb][32 * i : 32 * (i + 1), bass.ds(32 * i, 32)],
                        PtT[32 * i : 32 * (i + 1), :],
                    )

            # ================= pass 2: attn@v + proj + fusion =================
            p1stack.close()
            ps_av = ctx.enter_context(tc.tile_pool(name="ps_av", bufs=4, space="PSUM"))
            ps_po = ctx.enter_context(tc.tile_pool(name="ps_po", bufs=4, space="PSUM"))
            def p2_load_av(ch):
                n0 = 512 * ch
                xy_t = p2p.tile([128, 512], BF16, tag="xy", name="xy_t")
                z_t = p2p.tile([64, 512], BF16, tag="zt", name="z_t")
                nc.gpsimd.dma_start(out=xy_t[0:64, :], in_=xd[:, bass.ds(n0, 512)])
                nc.gpsimd.dma_start(out=xy_t[64:128, :], in_=yd[:, bass.ds(n0, 512)])
                nc.gpsimd.dma_start(out=z_t, in_=zd[:, bass.ds(n0, 512)])
                ao = []
                for mb in range(2):
                    pav = ps_av.tile([128, 512], F32, tag="pav", name="pav")
                    nc.tensor.matmul(
                        pav,
                        lhsT=bd[mb],
                        rhs=v_slab[mb][:, bass.ds(n0, 512)],
                        start=True,
                        stop=True,
                    )
                    ao_t = p2p.tile([128, 512], BF16, tag=f"ao{mb}", name="ao_t")
                    if mb == 0:
                        nc.scalar.copy(ao_t, pav)
                    else:
                        nc.vector.tensor_copy(ao_t, pav)
                    ao.append(ao_t)
                return xy_t, z_t, ao

            def p2_proj(ch, xy_t, z_t, ao):
                n0 = 512 * ch
                for mb in range(2):
                    po = ps_po.tile([128, 512], F32, tag="po", name="po")
                    nc.tensor.matmul(
                        po,
                        lhsT=wp_sb[:, bass.ds(0 * 256 + mb * 128, 128)],
                        rhs=ao[0],
                        start=True,
                        stop=False,
                    )
                    nc.tensor.matmul(
                        po,
                        lhsT=wp_sb[:, bass.ds(1 * 256 + mb * 128, 128)],
                        rhs=ao[1],
                        start=False,
                        stop=False,
                    )
                    nc.tensor.matmul(
                        po,
                        lhsT=wf_sb[0:128, bass.ds(mb * 128, 128)],
                        rhs=xy_t,
                        start=False,
                        stop=False,
                    )
                    nc.tensor.matmul(
                        po,
                        lhsT=wf_sb[0:64, bass.ds(256 + mb * 128, 128)],
                        rhs=z_t,
                        start=False,
                        stop=True,
                    )
                    o_t = p2p.tile([128, 512], F32, tag=f"ot{mb}", name="o_t")
                    if mb == 0:
                        nc.scalar.copy(o_t, po)
                    else:
                        nc.vector.tensor_copy(o_t, po)
                    (nc.sync if mb == 0 else nc.scalar).dma_start(
                        out=od[bass.ds(mb * 128, 128), bass.ds(n0, 512)], in_=o_t
                    )

            pending = None
            for ch in range(32):
                cur = p2_load_av(ch)
                if pending is not None:
                    p2_proj(ch - 1, *pending)
                pending = cur
            p2_proj(31, *pending)

    _split_excess_waits(nc)
    return nc


def kernel(**inputs):
    x = np.asarray(inputs["x"], np.float32)
    y = np.asarray(inputs["y"], np.float32)
    z = np.asarray(inputs["z"], np.float32)
    B = x.shape[0]
    assert B == 8

    wq = _merge_w(np.asarray(inputs["Wq"], np.float32), np.asarray(inputs["Wq_dw"], np.float32))
    wk = _merge_w(np.asarray(inputs["Wk"], np.float32), np.asarray(inputs["Wk_dw"], np.float32))
    wv = _merge_w(np.asarray(inputs["Wv"], np.float32), np.asarray(inputs["Wv_dw"], np.float32))

    wproj = np.asarray(inputs["Wproj"], np.float32)[:, :, 0, 0]  # [256,256] out,in
    wprojT = np.zeros((128, 512), np.float32)
    for kb in range(2):
        # [p, kb*256 + m] = Wproj[m, kb*128 + p]
        wprojT[:, kb * 256 : (kb + 1) * 256] = wproj[:, kb * 128 : (kb + 1) * 128].T

    wfus = np.asarray(inputs["Wfus"], np.float32)[:, :, 0, 0]  # [256, 192]
    wfusT = np.zeros((128, 512), np.float32)
    wfusT[:, 0:256] = wfus[:, 0:128].T          # x,y rows
    wfusT[0:64, 256:512] = wfus[:, 128:192].T   # z rows

    temp = np.asarray(inputs["temperature"], np.float32).reshape(HEADS)
    tfull = np.repeat(temp, 32).astype(np.float32)
    temp_cols = [tfull[0:128].reshape(128, 1), tfull[128:256].reshape(128, 1)]

    nc = _build_nc(wq, wk, wv, wprojT, wfusT, temp_cols)

    in_maps = []
    for i in range(B):
        in_maps.append(
            {
                "x": _bf(x[i].reshape(C, N)),
                "y": _bf(y[i].reshape(C, N)),
                "z": _bf(z[i].reshape(C, N)),
            }
        )
    res = run_bass_kernel_spmd(nc, in_maps, list(range(8)))
    out = np.stack(
        [np.asarray(res.results[i]["out"], np.float32).reshape(DIM, H, W) for i in range(B)]
    )
    return out



# revision 3
# speedup vs baseline: 2.1465x; 2.1465x over previous
"""Trainium2 Bass kernel for nn_LHFA_76278619177511.

Fused transposed-attention block (LHFA):
  q = dwconv3(conv1x1(x, Wq), Wq_dw)   (k from y, v from z)
  attn = softmax(l2norm(q) @ l2norm(k)^T * temp)   per-head [32,32]
  out = Wproj @ (attn @ v) + Wfus @ [x;y;z]

V2 strategy (pure DP over batch B=8 on 8 cores), key ideas:
  - fp8 DoubleRow matmuls (0.5 cycles/row, 2 K-tiles per instruction):
    the merged 1x1+dw conv (K=576) runs in 3 DR matmuls instead of 5
    bf16 passes. Pad row pitch = 144 so every DR pair stride/offset is
    16-aligned (dual-fp8 Ldweights ISA restriction).
  - q/k path entirely in fp8-e4m3 with weights pre-scaled x128; the
    scale cancels exactly in the L2 normalization.
  - The whole v path is folded into weights: W2T = (Wproj@attn@V9)^T is
    built on-device with tiny matmuls after softmax, then the attention
    output term becomes a single fused conv W2T^T @ z9 in fp8-e5m2
    accumulated INTO THE SAME PSUM as the bf16 fus conv (scales cancel:
    W2T stored x2^8, z fed x2^-8). No v slab, no attn@v, no proj.
  - Grams/sq-norms via DR on row pairs (64 pairs over the image).
  - attn output term is ~500x smaller than the fus term, so fp8 noise
    in it is invisible at the 2e-2 gate; fus stays bf16.
"""

import numpy as np
import ml_dtypes

import bass_rust
import concourse.bass as bass
import concourse.mybir as mybir
from concourse import tile as tile_mod
from concourse.tile import TileContext
from concourse.vector_clock import ScopedClock
from concourse.bass_utils import run_bass_kernel_spmd

BF16 = mybir.dt.bfloat16
F32 = mybir.dt.float32
E4 = mybir.dt.float8e4
E5 = mybir.dt.float8e5
NP_E4 = ml_dtypes.float8_e4m3
NP_E5 = ml_dtypes.float8_e5m2
NP_BF = ml_dtypes.bfloat16
DR = mybir.MatmulPerfMode.DoubleRow

C = 64          # input channels
DIM = 256       # q/k/v channels
HEADS = 8
H = W = 128
N = H * W       # 16384
PW = 144        # padded row pitch (16-aligned; image cols at [1,129))
HB = 16         # band height
NB = H // HB    # 8 bands
NRT = HB + 2    # rows per band tile
TWA = NRT * PW  # 2592: region width (AB / AD each)
ADB = 2606      # AD region base col (== 14 mod 16 so P1 stride is 16-mult)
FA = 7168       # pad tile alloc width (slice-bound slack)

# DR pair definitions: (X0(base), delta). base = hl*PW.
#   P0: (S0 @ AB+base,        S1 @ AB+base+PW)        taps rows -1, 0
#   P1: (S2 @ AB+base+2PW,    S3 @ AD+base+2)         taps row +1, col +1
#   P2: (dummy @ AD+base+2,   S4 @ AD+base+2PW+2)     tap (1,1) (A only)
P0D = PW
P1D = ADB + 2 - 2 * PW   # 2320
P2D = 2 * PW             # 288
assert P1D % 16 == 0 and ADB % 16 == 14
# weight slots [6 x 256]: 0=S0 1=S1 2=S2 3=S3 4=ZERO 5=S4
# taps (dy,dx) per slot/half for weight merging (half 0 = A rows,
# half 1 = B rows (slots 0-2, B = A<<1col) or D rows (slot 3, D = A<<1row))
SLOT_TAPS = [
    [(-1, -1), (-1, 0)],
    [(0, -1), (0, 0)],
    [(1, -1), (1, 0)],
    [(-1, 1), (0, 1)],
    [],
    [(1, 1)],
]

QK_SCALE = 128.0      # q/k conv weight prescale (cancels in l2norm)
V9_SCALE = 2.0 ** 12  # V9 prescale into e4m3
PR_SCALE = 2.0 ** 6   # WprojN prescale into e4m3
AT_SCALE = 32.0       # attn (Pt) prescale into e4m3
W2_OUT_SCALE = 2.0 ** 8   # W2T stored scale (z fed at 2^-8)
Z8_SCALE = 2.0 ** -8
# W2 psum carries AT*V9*PR = 2^23; store at 2^8 -> copy scale 2^-15
W2_COPY_SCALE = W2_OUT_SCALE / (AT_SCALE * V9_SCALE * PR_SCALE)

_PATCHED = False


def _patch_tile_drain():
    """This walrus build rejects >1 sem wait on a CTRL (Drain) instruction;
    split the TileContext tail-drain waits onto individual nops."""
    global _PATCHED
    if _PATCHED:
        return
    _PATCHED = True

    def _drain_and_barrier(self, tick_clock, wait_clock):
        nc = self.nc
        drain_inst = nc.sync.drain()
        wait_clock.add_sem_waits(
            drain_inst.ins, ScopedClock({None: tick_clock.global_clock})
        )
        si = drain_inst.ins.sync_info
        waits = list(si.on_wait or [])
        if len(waits) > 1:
            si.on_wait = waits[:1]
            for w in waits[1:]:
                nop = nc.sync.nop(nofuse=True, hint="split_wait")
                nop.ins.sync_info = bass_rust.SyncInfo(on_wait=[w], on_update=[])
        nc.all_engine_barrier()
        assert self.sems is not None
        popped = nc._tile_sem_poison_stack.pop()
        assert popped is self._sem_poison
        nc.clear_and_free_semaphores(list(self.sems.allocated().values()))
        nc.all_engine_barrier()

    tile_mod.TileContext._drain_and_barrier = _drain_and_barrier
    try:
        from concourse import tile_utils
        tile_utils.max_sbuf_usage = 208 * 1024
    except Exception:
        pass


def _split_excess_waits(nc, max_waits=1):
    """This walrus build caps sem waits per instruction encoding; hoist
    excess waits onto preceding same-engine NoOps (queues are in-order,
    so a wait satisfied on an earlier instruction orders the later one)."""
    import bass_rust as _br

    ctr = [0]
    for f in nc.m.functions:
        for blk in f.blocks:
            out = []
            for inst in blk.instructions:
                si = inst.sync_info
                waits = list(si.on_wait) if (si and si.on_wait) else []
                if len(waits) > max_waits:
                    keep = waits[:max_waits]
                    extra = waits[max_waits:]
                    si.on_wait = keep
                    for w in extra:
                        ctr[0] += 1
                        nop = _br.InstNoOp(name=f"wsplit-{ctr[0]}", ins=[], outs=[])
                        nop.engine = inst.engine
                        nop.sync_info = _br.SyncInfo(on_wait=[w], on_update=[])
                        try:
                            nc.register_instruction(nop, overwrite=True)
                        except Exception:
                            pass
                        out.append(nop)
                out.append(inst)
            blk.instructions[:] = out


def _merge_w(W1, Wdw, scale):
    """-> [128, 6, 256] float32 merged conv weights in slot layout.
    [p=(half,chan), slot, outch] = dw[out, tap(slot, half)] * W1[out, chan]."""
    out = np.zeros((128, 6, 256), np.float32)
    W1 = W1[:, :, 0, 0]  # [256, 64]
    for s, taps in enumerate(SLOT_TAPS):
        for half, (dy, dx) in enumerate(taps):
            hh = half if s != 5 else 0
            out[hh * 64: (hh + 1) * 64, s, :] = (
                Wdw[:, 0, 1 + dy, 1 + dx][:, None] * W1
            ).T * scale
    return out


def _merge_v9(W1, Wdw, scale):
    """-> [256, 576] float32: V9[d, s*128 + half*64 + c] for slots 0-3 +
    slot5(A half) packed as k-chunk order s in 0..4 (chunk4 = slot5 taps)."""
    out = np.zeros((256, 640), np.float32)
    W1 = W1[:, :, 0, 0]  # [256, 64]
    for s, taps in enumerate(SLOT_TAPS):
        if s == 4:
            continue
        kc = s if s < 4 else 4
        for half, (dy, dx) in enumerate(taps):
            hh = half if s != 5 else 0
            out[:, kc * 128 + hh * 64: kc * 128 + (hh + 1) * 64] = (
                Wdw[:, 0, 1 + dy, 1 + dx][:, None] * W1
            ) * scale
    return out[:, :576]


def _bf(a):
    return np.ascontiguousarray(a).astype(NP_BF)


def _pair_ap(tile_ap, X, delta, width):
    """[128, 2, width] AP: members at cols X and X+delta."""
    return tile_ap[:, X: X + 2 * delta].rearrange(
        "p (two m) -> p two m", two=2
    )[:, :, 0:width]


def _build_nc(wq, wk, v9, wprojN, wfusT, temp_cols):
    """Build the Bass module. Weight arrays pre-merged/scaled fp32."""
    _patch_tile_drain()
    nc = bass.Bass()

    xd = nc.declare_dram_parameter("x", [C, N], BF16, isOutput=False)
    yd = nc.declare_dram_parameter("y", [C, N], BF16, isOutput=False)
    zd = nc.declare_dram_parameter("z", [C, N], BF16, isOutput=False)
    x8d = nc.declare_dram_parameter("x8", [C, N], E4, isOutput=False)
    y8d = nc.declare_dram_parameter("y8", [C, N], E4, isOutput=False)
    z8d = nc.declare_dram_parameter("z8", [C, N], E5, isOutput=False)
    od = nc.declare_dram_parameter("out", [DIM, N], F32, isOutput=True)

    wq_d = nc.inline_tensor(
        np.ascontiguousarray(wq.reshape(128, 6 * 256)).astype(NP_E4), name="wq9")
    wk_d = nc.inline_tensor(
        np.ascontiguousarray(wk.reshape(128, 6 * 256)).astype(NP_E4), name="wk9")
    # V9 [256,576] -> 2 mb tiles side by side [128, 1152]
    v9_2 = np.concatenate([v9[0:128], v9[128:256]], axis=1)
    v9_d = nc.inline_tensor(np.ascontiguousarray(v9_2).astype(NP_E4), name="v9")
    wp_d = nc.inline_tensor(
        np.ascontiguousarray(wprojN).astype(NP_E4), name="wprojN")  # [128, 512]
    wf_d = nc.inline_tensor(_bf(wfusT), name="wfusT")               # [128, 512]
    tc0_d = nc.inline_tensor(np.ascontiguousarray(temp_cols[0]), name="tcol0")
    tc1_d = nc.inline_tensor(np.ascontiguousarray(temp_cols[1]), name="tcol1")
    id_d = nc.inline_tensor(np.eye(128, dtype=NP_BF), name="ident")

    with TileContext(nc) as tc:
        import contextlib

        with contextlib.ExitStack() as ctx:
            wpool = ctx.enter_context(tc.tile_pool(name="wpool", bufs=1))
            pads = ctx.enter_context(tc.tile_pool(name="pads", bufs=2))
            qkp = ctx.enter_context(tc.tile_pool(name="qkp", bufs=3))
            smallp = ctx.enter_context(tc.tile_pool(name="smallp", bufs=2))

            # --- weights to SBUF ---
            wq_sb = wpool.tile([128, 6 * 256], E4, tag="wq")
            wk_sb = wpool.tile([128, 6 * 256], E4, tag="wk")
            v9_sb = wpool.tile([128, 2 * 576], E4, tag="v9")
            wp_sb = wpool.tile([128, 512], E4, tag="wp")
            wf_sb = wpool.tile([128, 512], BF16, tag="wf")
            w2t_sb = wpool.tile([128, 6 * 256], E5, tag="w2t")
            ident_sb = wpool.tile([128, 128], BF16, tag="ident")
            tcol = [wpool.tile([128, 1], F32, tag=f"tc{i}", name=f"tcol{i}")
                    for i in range(2)]
            nc.scalar.dma_start(out=wq_sb, in_=wq_d[:])
            nc.gpsimd.dma_start(out=wk_sb, in_=wk_d[:])
            nc.sync.dma_start(out=v9_sb, in_=v9_d[:])
            nc.sync.dma_start(out=wp_sb, in_=wp_d[:])
            nc.sync.dma_start(out=wf_sb, in_=wf_d[:])
            nc.sync.dma_start(out=tcol[0], in_=tc0_d[:])
            nc.sync.dma_start(out=tcol[1], in_=tc1_d[:])
            nc.sync.dma_start(out=ident_sb, in_=id_d[:])

            # --- pass-1 psums ---
            p1stack = ctx.enter_context(contextlib.ExitStack())
            ps_qk = p1stack.enter_context(
                tc.tile_pool(name="ps_qk", bufs=3, space="PSUM"))
            ps_acc = p1stack.enter_context(
                tc.tile_pool(name="ps_acc", bufs=1, space="PSUM"))
            acc1 = ps_acc.tile([128, 512], F32, tag="acc1")
            acc2 = ps_acc.tile([128, 256], F32, tag="acc2")
            par_all = acc1[:, 0:256]
            pgq = acc1[:, 256:512]
            pgk = acc2

            def build_pads(pool, dram8, name, band, dt8):
                """Build the [128, FA] padded AB|AD tile for one band."""
                P = pool.tile([128, FA], dt8, tag=f"pad{name}")
                lr0 = 1 if band == 0 else 0
                nr = NRT - (1 if band == 0 else 0) - (1 if band == NB - 1 else 0)
                ir0 = max(0, HB * band - 1)
                src_img = dram8[:].rearrange("p (r c) -> p r c", c=W)[
                    :, ir0: ir0 + nr, :]
                ap = P[:]
                for rb in (0, ADB):
                    view = ap[0:64, rb: rb + TWA].rearrange(
                        "p (r c) -> p r c", c=PW)
                    nc.gpsimd.memset(view[:, :, 0:1], 0.0)
                    nc.gpsimd.memset(view[:, :, 129:PW], 0.0)
                    if band == 0:
                        nc.gpsimd.memset(view[:, 0:1, :], 0.0)
                    if band == NB - 1:
                        nc.gpsimd.memset(view[:, NRT - 1: NRT, :], 0.0)
                    eng = nc.sync if rb == 0 else nc.gpsimd
                    eng.dma_start(
                        out=view[:, lr0: lr0 + nr, 1: 1 + W], in_=src_img)
                # B = A << 1 col (AB region)
                nc.sync.dma_start(
                    out=ap[64:128, 0: TWA - 1], in_=ap[0:64, 1: TWA])
                # D = A << 1 row (AD region); zero the D tail rows
                nc.gpsimd.dma_start(
                    out=ap[64:128, ADB: ADB + TWA - PW],
                    in_=ap[0:64, ADB + PW: ADB + TWA])
                nc.gpsimd.memset(ap[64:128, ADB + TWA - PW: ADB + TWA], 0.0)
                return P

            def conv_drs(P, w_sb, pt, base, start_tag):
                """3 DR matmuls accumulating one row's conv into pt [128,256]."""
                ap = P[:]
                w3 = w_sb[:].rearrange("p (s n) -> p s n", n=256)
                pairs = [
                    (base, P0D, 0),
                    (base + 2 * PW, P1D, 2),
                    (ADB + base + 2, P2D, 4),
                ]
                for j, (X, D_, ws) in enumerate(pairs):
                    nc.tensor.matmul(
                        pt,
                        lhsT=_pair_ap(ap, X, D_, 128),
                        rhs=w3[:, ws: ws + 2, :],
                        start=(j == 0),
                        stop=(j == 2),
                        perf_mode=DR,
                    )

            # ================= pass 1: q/k convs + grams =================
            pend_gram = None  # (cat tile, first, last)

            def emit_grams(cat, first, last):
                cat3 = cat[:].rearrange("p (two c) -> p two c", two=2)
                for mb in range(2):
                    qsl = cat3[:, :, 256 + 128 * mb: 256 + 128 * mb + 128]
                    ksl = cat3[:, :, 128 * mb: 128 * mb + 128]
                    nc.tensor.matmul(
                        par_all[:, bass.ds(mb * 128, 128)],
                        lhsT=qsl, rhs=ksl, start=first, stop=last,
                        perf_mode=DR, skip_group_check=True)
                    nc.tensor.matmul(
                        pgq[:, bass.ds(mb * 128, 128)],
                        lhsT=qsl, rhs=qsl, start=first, stop=last,
                        perf_mode=DR, skip_group_check=True)
                    nc.tensor.matmul(
                        pgk[:, bass.ds(mb * 128, 128)],
                        lhsT=ksl, rhs=ksl, start=first, stop=last,
                        perf_mode=DR, skip_group_check=True)

            for b in range(NB):
                xP = build_pads(pads, x8d, "x", b, E4)
                yP = build_pads(pads, y8d, "y", b, E4)
                for hl2 in range(HB // 2):
                    pqk = ps_qk.tile([128, 1024], F32, tag="pqk")
                    for half in range(2):  # even/odd row of the pair
                        base = (2 * hl2 + half) * PW
                        o = 512 * half
                        conv_drs(yP, wk_sb, pqk[:, o: o + 256], base, "k")
                        conv_drs(xP, wq_sb, pqk[:, o + 256: o + 512], base, "q")
                    cat = qkp.tile([128, 1024], E4, tag="cat")
                    nc.scalar.copy(cat[:, 0:512], pqk[:, 0:512])
                    nc.vector.tensor_copy(cat[:, 512:1024], pqk[:, 512:1024])
                    if pend_gram is not None:
                        emit_grams(*pend_gram)
                    pg = 8 * b + hl2
                    pend_gram = (cat, pg == 0, pg == 63)
            emit_grams(*pend_gram)

            # ================= softmax on per-head [32,32] =================
            ar_sb = [smallp.tile([128, 128], F32, tag=f"arsb{mb}",
                                 name=f"arsb{mb}") for mb in range(2)]
            nc.scalar.copy(ar_sb[0], par_all[:, 0:128])
            nc.scalar.copy(ar_sb[1], par_all[:, 128:256])
            bd8 = [smallp.tile([128, 128], E4, tag=f"bd{mb}",
                               name=f"bdiag{mb}") for mb in range(2)]
            for mb in range(2):
                scr = smallp.tile([128, 128], F32, tag="scr")
                rnq_c = smallp.tile([128, 1], F32, tag="rnq")
                rnk_c = smallp.tile([128, 1], F32, tag="rnk")
                for g_ps, dst in ((pgq, rnq_c), (pgk, rnk_c)):
                    ssum = smallp.tile([128, 1], F32, tag="ssum")
                    nc.vector.tensor_mul(
                        scr, g_ps[:, bass.ds(mb * 128, 128)], ident_sb)
                    nc.vector.reduce_sum(
                        out=ssum, in_=scr, axis=mybir.AxisListType.X)
                    nc.scalar.sqrt(ssum, ssum)
                    nc.vector.tensor_scalar_max(ssum, ssum, 1e-12)
                    nc.vector.reciprocal(dst, ssum)
                rnqt = smallp.tile([128, 1], F32, tag="rnqt")
                nc.vector.tensor_mul(rnqt, rnq_c, tcol[mb])

                hd = smallp.tile([128, 32], F32, tag="hd")
                for i in range(4):
                    nc.vector.tensor_copy(
                        hd[32 * i: 32 * (i + 1), :],
                        ar_sb[mb][32 * i: 32 * (i + 1), bass.ds(32 * i, 32)],
                    )
                hds = smallp.tile([128, 32], F32, tag="hds")
                nc.scalar.activation(
                    hds, hd, mybir.ActivationFunctionType.Copy,
                    bias=0.0, scale=rnqt)
                hdT = smallp.tile([128, 32], F32, tag="hdT")
                nc.vector.transpose(hdT, hds)
                hdTs = smallp.tile([128, 32], F32, tag="hdTs")
                nc.scalar.activation(
                    hdTs, hdT, mybir.ActivationFunctionType.Copy,
                    bias=0.0, scale=rnk_c)
                hd3 = smallp.tile([128, 32], F32, tag="hd3")
                nc.vector.transpose(hd3, hdTs)
                nmx = smallp.tile([128, 1], F32, tag="nmx")
                nc.vector.reduce_max(
                    out=nmx, in_=hd3, axis=mybir.AxisListType.X, negate=True)
                ex = smallp.tile([128, 32], F32, tag="ex")
                nc.scalar.activation(
                    ex, hd3, mybir.ActivationFunctionType.Exp,
                    bias=nmx, scale=1.0)
                sm = smallp.tile([128, 1], F32, tag="sm")
                nc.vector.reduce_sum(out=sm, in_=ex, axis=mybir.AxisListType.X)
                rsm = smallp.tile([128, 1], F32, tag="rsm")
                nc.vector.reciprocal(rsm, sm)
                rsm32 = smallp.tile([128, 1], F32, tag="rsm32")
                nc.scalar.activation(
                    rsm32, rsm, mybir.ActivationFunctionType.Copy,
                    bias=0.0, scale=AT_SCALE)
                Pt = smallp.tile([128, 32], F32, tag="Pt")
                nc.scalar.activation(
                    Pt, ex, mybir.ActivationFunctionType.Copy,
                    bias=0.0, scale=rsm32)
                PtT = smallp.tile([128, 32], F32, tag="PtT")
                nc.vector.transpose(PtT, Pt)
                nc.gpsimd.memset(bd8[mb], 0.0)
                for i in range(4):
                    nc.vector.tensor_copy(
                        bd8[mb][32 * i: 32 * (i + 1), bass.ds(32 * i, 32)],
                        PtT[32 * i: 32 * (i + 1), :],
                    )

            # ================= W2T build =================
            p1stack.close()
            wbstack = ctx.enter_context(contextlib.ExitStack())
            ps_w1 = wbstack.enter_context(
                tc.tile_pool(name="ps_w1", bufs=2, space="PSUM"))
            ps_w2 = wbstack.enter_context(
                tc.tile_pool(name="ps_w2", bufs=2, space="PSUM"))
            w1_sb = smallp.tile([128, 2 * 576], E4, tag="w1sb", name="w1sb")
            for mb in range(2):
                w1ps = ps_w1.tile([128, 576], F32, tag="w1ps")
                vsl = v9_sb[:, 576 * mb: 576 * mb + 576]
                nc.tensor.matmul(w1ps[:, 0:512], lhsT=bd8[mb],
                                 rhs=vsl[:, 0:512], start=True, stop=True)
                nc.tensor.matmul(w1ps[:, 512:576], lhsT=bd8[mb],
                                 rhs=vsl[:, 512:576], start=True, stop=True)
                (nc.scalar.copy if mb == 0 else nc.vector.tensor_copy)(
                    w1_sb[:, 576 * mb: 576 * mb + 576], w1ps)
            w13 = w1_sb[:].rearrange("p (two k) -> p two k", two=2)
            wp3 = wp_sb[:].rearrange("p (two n) -> p two n", two=2)
            # slot 4 of w2t is zeros; k-chunk 4 goes to slot 5
            nc.gpsimd.memset(w2t_sb[:, 4 * 256: 5 * 256], 0.0)
            nc.gpsimd.memset(w2t_sb[64:128, 5 * 256: 6 * 256], 0.0)
            for j in range(5):
                kw = 128 if j < 4 else 64
                w2ps = ps_w2.tile([128, 256], F32, tag="w2ps")
                nc.tensor.matmul(
                    w2ps[0:kw, :],
                    lhsT=w13[:, :, 128 * j: 128 * j + kw],
                    rhs=wp3, start=True, stop=True, perf_mode=DR)
                slot = j if j < 4 else 5
                nc.scalar.activation(
                    w2t_sb[0:kw, slot * 256: (slot + 1) * 256], w2ps[0:kw, :],
                    mybir.ActivationFunctionType.Copy,
                    bias=0.0, scale=float(W2_COPY_SCALE))

            # ================= phase 2: fus + W2T@z9 =================
            wbstack.close()
            p2p = ctx.enter_context(tc.tile_pool(name="p2p", bufs=3))
            ps_o = ctx.enter_context(
                tc.tile_pool(name="ps_o", bufs=4, space="PSUM"))
            w2t3 = w2t_sb[:].rearrange("p (s n) -> p s n", n=256)

            zP = None
            for bz in range(NB):
                zP = build_pads(pads, z8d, "z", bz, E5)
                zap = zP[:]
                for cc in range(4):
                    g = 4 * bz + cc
                    n0 = 512 * g
                    xy_t = p2p.tile([128, 512], BF16, tag="xy")
                    z_t = p2p.tile([64, 512], BF16, tag="zt")
                    nc.gpsimd.dma_start(out=xy_t[0:64, :],
                                        in_=xd[:, bass.ds(n0, 512)])
                    nc.gpsimd.dma_start(out=xy_t[64:128, :],
                                        in_=yd[:, bass.ds(n0, 512)])
                    nc.gpsimd.dma_start(out=z_t, in_=zd[:, bass.ds(n0, 512)])
                    for mb in range(2):
                        po = ps_o.tile([128, 512], F32, tag="po")
                        nc.tensor.matmul(
                            po, lhsT=wf_sb[:, bass.ds(mb * 128, 128)],
                            rhs=xy_t, start=True, stop=False)
                        nc.tensor.matmul(
                            po, lhsT=wf_sb[0:64, bass.ds(256 + mb * 128, 128)],
                            rhs=z_t, start=False, stop=False)
                        for r in range(4):
                            base = (4 * cc + r) * PW
                            pairs = [
                                (base, P0D, 0),
                                (base + 2 * PW, P1D, 2),
                                (ADB + base + 2, P2D, 4),
                            ]
                            for j, (X, D_, ws) in enumerate(pairs):
                                nc.tensor.matmul(
                                    po[:, 128 * r: 128 * r + 128],
                                    lhsT=w2t3[:, ws: ws + 2,
                                              128 * mb: 128 * mb + 128],
                                    rhs=_pair_ap(zap, X, D_, 128),
                                    start=False,
                                    stop=(r == 3 and j == 2),
                                    perf_mode=DR,
                                    skip_group_check=True)
                        o_t = p2p.tile([128, 512], F32, tag=f"ot{mb}",
                                       name="o_t")
                        if mb == 0:
                            nc.scalar.copy(o_t, po)
                        else:
                            nc.vector.tensor_copy(o_t, po)
                        (nc.sync if mb == 0 else nc.scalar).dma_start(
                            out=od[bass.ds(mb * 128, 128), bass.ds(n0, 512)],
                            in_=o_t)

    _split_excess_waits(nc)
    return nc


def _prep_weights(inputs):
    wq = _merge_w(np.asarray(inputs["Wq"], np.float32),
                  np.asarray(inputs["Wq_dw"], np.float32), QK_SCALE)
    wk = _merge_w(np.asarray(inputs["Wk"], np.float32),
                  np.asarray(inputs["Wk_dw"], np.float32), QK_SCALE)
    v9 = _merge_v9(np.asarray(inputs["Wv"], np.float32),
                   np.asarray(inputs["Wv_dw"], np.float32), V9_SCALE)

    wproj = np.asarray(inputs["Wproj"], np.float32)[:, :, 0, 0]  # [256,256]
    # WprojN [c, o] mb tiles side by side: [128, 512]
    wprojN = np.zeros((128, 512), np.float32)
    wprojN[:, 0:256] = wproj[:, 0:128].T * PR_SCALE
    wprojN[:, 256:512] = wproj[:, 128:256].T * PR_SCALE

    wfus = np.asarray(inputs["Wfus"], np.float32)[:, :, 0, 0]  # [256, 192]
    wfusT = np.zeros((128, 512), np.float32)
    wfusT[:, 0:256] = wfus[:, 0:128].T          # x,y rows
    wfusT[0:64, 256:512] = wfus[:, 128:192].T   # z rows

    temp = np.asarray(inputs["temperature"], np.float32).reshape(HEADS)
    tfull = np.repeat(temp, 32).astype(np.float32)
    temp_cols = [tfull[0:128].reshape(128, 1), tfull[128:256].reshape(128, 1)]
    return wq, wk, v9, wprojN, wfusT, temp_cols


def kernel(**inputs):
    x = np.asarray(inputs["x"], np.float32)
    y = np.asarray(inputs["y"], np.float32)
    z = np.asarray(inputs["z"], np.float32)
    B = x.shape[0]
    assert B == 8

    nc = _build_nc(*_prep_weights(inputs))

    in_maps = []
    for i in range(B):
        xi = x[i].reshape(C, N)
        yi = y[i].reshape(C, N)
        zi = z[i].reshape(C, N)
        in_maps.append({
            "x": _bf(xi),
            "y": _bf(yi),
            "z": _bf(zi),
            "x8": np.ascontiguousarray(xi).astype(NP_E4),
            "y8": np.ascontiguousarray(yi).astype(NP_E4),
            "z8": np.ascontiguousarray(zi * Z8_SCALE).astype(NP_E5),
        })
    res = run_bass_kernel_spmd(nc, in_maps, list(range(8)))
    out = np.stack(
        [np.asarray(res.results[i]["out"], np.float32).reshape(DIM, H, W)
         for i in range(B)]
    )
    return out


# revision 16
# speedup vs baseline: 2.4126x; 1.1240x over previous
"""Trainium2 Bass kernel for nn_LHFA_76278619177511.

Fused transposed-attention block (LHFA):
  q = dwconv3(conv1x1(x, Wq), Wq_dw)   (k from y, v from z)
  attn = softmax(l2norm(q) @ l2norm(k)^T * temp)   per-head [32,32]
  out = Wproj @ (attn @ v) + Wfus @ [x;y;z]

V2 strategy (pure DP over batch B=8 on 8 cores), key ideas:
  - fp8 DoubleRow matmuls (0.5 cycles/row, 2 K-tiles per instruction):
    the merged 1x1+dw conv (K=576) runs in 3 DR matmuls instead of 5
    bf16 passes. Pad row pitch = 144 so every DR pair stride/offset is
    16-aligned (dual-fp8 Ldweights ISA restriction).
  - q/k path entirely in fp8-e4m3 with weights pre-scaled x128; the
    scale cancels exactly in the L2 normalization.
  - The whole v path is folded into weights: W2T = (Wproj@attn@V9)^T is
    built on-device with tiny matmuls after softmax, then the attention
    output term becomes a single fused conv W2T^T @ z9 in fp8-e5m2
    accumulated INTO THE SAME PSUM as the bf16 fus conv (scales cancel:
    W2T stored x2^8, z fed x2^-8). No v slab, no attn@v, no proj.
  - Grams/sq-norms via DR on row pairs (64 pairs over the image).
  - attn output term is ~500x smaller than the fus term, so fp8 noise
    in it is invisible at the 2e-2 gate; fus stays bf16.
"""

import numpy as np
import ml_dtypes

import bass_rust
import concourse.bass as bass
import concourse.mybir as mybir
from concourse import tile as tile_mod
from concourse.tile import TileContext
from concourse.vector_clock import ScopedClock
from concourse.bass_utils import run_bass_kernel_spmd

BF16 = mybir.dt.bfloat16
F32 = mybir.dt.float32
E4 = mybir.dt.float8e4
E5 = mybir.dt.float8e5
NP_E4 = ml_dtypes.float8_e4m3
NP_E5 = ml_dtypes.float8_e5m2
NP_BF = ml_dtypes.bfloat16
DR = mybir.MatmulPerfMode.DoubleRow

C = 64          # input channels
DIM = 256       # q/k/v channels
HEADS = 8
H = W = 128
N = H * W       # 16384
PW = 144        # padded row pitch (16-aligned; image cols at [1,129))
HB = 16         # band height
NB = H // HB    # 8 bands
NRT = HB + 2    # rows per band tile
TWA = NRT * PW  # 2592: region width (AB / AD each)
ADB = 2606      # AD region base col (== 14 mod 16 so P1 stride is 16-mult)
FA = 7168       # pad tile alloc width (slice-bound slack)

# DR pair definitions: (X0(base), delta). base = hl*PW.
#   P0: (S0 @ AB+base,        S1 @ AB+base+PW)        taps rows -1, 0
#   P1: (S2 @ AB+base+2PW,    S3 @ AD+base+2)         taps row +1, col +1
#   P2: (dummy @ AD+base+2,   S4 @ AD+base+2PW+2)     tap (1,1) (A only)
P0D = PW
P1D = ADB + 2 - 2 * PW   # 2320
P2D = 2 * PW             # 288
assert P1D % 16 == 0 and ADB % 16 == 14
# weight slots [6 x 256]: 0=S0 1=S1 2=S2 3=S3 4=ZERO 5=S4
# taps (dy,dx) per slot/half for weight merging (half 0 = A rows,
# half 1 = B rows (slots 0-2, B = A<<1col) or D rows (slot 3, D = A<<1row))
SLOT_TAPS = [
    [(-1, -1), (-1, 0)],
    [(0, -1), (0, 0)],
    [(1, -1), (1, 0)],
    [(-1, 1), (0, 1)],
    [],
    [(1, 1)],
]

QK_SCALE = 128.0      # q/k conv weight prescale (cancels in l2norm)
V9_SCALE = 2.0 ** 12  # V9 prescale into e4m3
PR_SCALE = 2.0 ** 6   # WprojN prescale into e4m3
AT_SCALE = 32.0       # attn (Pt) prescale into e4m3
W2_OUT_SCALE = 2.0 ** 8   # W2T stored scale (z fed at 2^-8)
Z8_SCALE = 2.0 ** -8
# W2 psum carries AT*V9*PR = 2^23; store at 2^8 -> copy scale 2^-15
W2_COPY_SCALE = W2_OUT_SCALE / (AT_SCALE * V9_SCALE * PR_SCALE)

_PATCHED = False


def _patch_tile_drain():
    """This walrus build rejects >1 sem wait on a CTRL (Drain) instruction;
    split the TileContext tail-drain waits onto individual nops."""
    global _PATCHED
    if _PATCHED:
        return
    _PATCHED = True

    def _drain_and_barrier(self, tick_clock, wait_clock):
        nc = self.nc
        drain_inst = nc.sync.drain()
        wait_clock.add_sem_waits(
            drain_inst.ins, ScopedClock({None: tick_clock.global_clock})
        )
        si = drain_inst.ins.sync_info
        waits = list(si.on_wait or [])
        if len(waits) > 1:
            si.on_wait = waits[:1]
            for w in waits[1:]:
                nop = nc.sync.nop(nofuse=True, hint="split_wait")
                nop.ins.sync_info = bass_rust.SyncInfo(on_wait=[w], on_update=[])
        nc.all_engine_barrier()
        assert self.sems is not None
        popped = nc._tile_sem_poison_stack.pop()
        assert popped is self._sem_poison
        nc.clear_and_free_semaphores(list(self.sems.allocated().values()))
        nc.all_engine_barrier()

    tile_mod.TileContext._drain_and_barrier = _drain_and_barrier
    try:
        from concourse import tile_utils
        tile_utils.max_sbuf_usage = 208 * 1024
    except Exception:
        pass


def _split_excess_waits(nc, max_waits=1):
    """This walrus build caps sem waits per instruction encoding; hoist
    excess waits onto preceding same-engine NoOps (queues are in-order,
    so a wait satisfied on an earlier instruction orders the later one)."""
    import bass_rust as _br

    ctr = [0]
    for f in nc.m.functions:
        for blk in f.blocks:
            out = []
            for inst in blk.instructions:
                si = inst.sync_info
                waits = list(si.on_wait) if (si and si.on_wait) else []
                if len(waits) > max_waits:
                    keep = waits[:max_waits]
                    extra = waits[max_waits:]
                    si.on_wait = keep
                    for w in extra:
                        ctr[0] += 1
                        nop = _br.InstNoOp(name=f"wsplit-{ctr[0]}", ins=[], outs=[])
                        nop.engine = inst.engine
                        nop.sync_info = _br.SyncInfo(on_wait=[w], on_update=[])
                        try:
                            nc.register_instruction(nop, overwrite=True)
                        except Exception:
                            pass
                        out.append(nop)
                out.append(inst)
            blk.instructions[:] = out


def _merge_w(W1, Wdw, scale):
    """-> [128, 6, 256] float32 merged conv weights in slot layout.
    [p=(half,chan), slot, outch] = dw[out, tap(slot, half)] * W1[out, chan]."""
    out = np.zeros((128, 6, 256), np.float32)
    W1 = W1[:, :, 0, 0]  # [256, 64]
    for s, taps in enumerate(SLOT_TAPS):
        for half, (dy, dx) in enumerate(taps):
            hh = half if s != 5 else 0
            out[hh * 64: (hh + 1) * 64, s, :] = (
                Wdw[:, 0, 1 + dy, 1 + dx][:, None] * W1
            ).T * scale
    return out


def _merge_v9(W1, Wdw, scale):
    """-> [256, 576] float32: V9[d, s*128 + half*64 + c] for slots 0-3 +
    slot5(A half) packed as k-chunk order s in 0..4 (chunk4 = slot5 taps)."""
    out = np.zeros((256, 640), np.float32)
    W1 = W1[:, :, 0, 0]  # [256, 64]
    for s, taps in enumerate(SLOT_TAPS):
        if s == 4:
            continue
        kc = s if s < 4 else 4
        for half, (dy, dx) in enumerate(taps):
            hh = half if s != 5 else 0
            out[:, kc * 128 + hh * 64: kc * 128 + (hh + 1) * 64] = (
                Wdw[:, 0, 1 + dy, 1 + dx][:, None] * W1
            ) * scale
    return out[:, :576]


def _bf(a):
    return np.ascontiguousarray(a).astype(NP_BF)


def _pair_ap(tile_ap, X, delta, width):
    """[128, 2, width] AP: members at cols X and X+delta."""
    return tile_ap[:, X: X + 2 * delta].rearrange(
        "p (two m) -> p two m", two=2
    )[:, :, 0:width]


def _build_nc(wq, wk, v9, wprojN, wfusT, temp_cols):
    """Build the Bass module. Weight arrays pre-merged/scaled fp32."""
    _patch_tile_drain()
    nc = bass.Bass()

    xd = nc.declare_dram_parameter("x", [C, N], BF16, isOutput=False)
    yd = nc.declare_dram_parameter("y", [C, N], BF16, isOutput=False)
    zd = nc.declare_dram_parameter("z", [C, N], BF16, isOutput=False)
    # pre-padded, pre-shifted canvases: [128, 130*PW]; parts 0:64 = A
    # (padded image), parts 64:128 = B (A<<1col) / D (A<<1row)
    LC = 130 * PW
    pad_d = {}
    for nm, dt8 in (("xab", E4), ("xad", E4), ("yab", E4), ("yad", E4),
                    ("zab", E5), ("zad", E5)):
        pad_d[nm] = nc.declare_dram_parameter(nm, [128, LC], dt8, isOutput=False)
    od = nc.declare_dram_parameter("out", [DIM, N], BF16, isOutput=True)

    wq_d = nc.inline_tensor(
        np.ascontiguousarray(wq.reshape(128, 6 * 256)).astype(NP_E4), name="wq9")
    wk_d = nc.inline_tensor(
        np.ascontiguousarray(wk.reshape(128, 6 * 256)).astype(NP_E4), name="wk9")
    # V9 [256,576] -> 2 mb tiles side by side [128, 1152]
    v9_2 = np.concatenate([v9[0:128], v9[128:256]], axis=1)
    v9_d = nc.inline_tensor(np.ascontiguousarray(v9_2).astype(NP_E4), name="v9")
    wp_d = nc.inline_tensor(
        np.ascontiguousarray(wprojN).astype(NP_E4), name="wprojN")  # [128, 512]
    wf_d = nc.inline_tensor(_bf(wfusT), name="wfusT")               # [128, 512]
    tc0_d = nc.inline_tensor(np.ascontiguousarray(temp_cols[0]), name="tcol0")
    tc1_d = nc.inline_tensor(np.ascontiguousarray(temp_cols[1]), name="tcol1")
    id_d = nc.inline_tensor(np.eye(128, dtype=NP_BF), name="ident")

    with TileContext(nc) as tc:
        import contextlib

        with contextlib.ExitStack() as ctx:
            wpool = ctx.enter_context(tc.tile_pool(name="wpool", bufs=1))
            pads = ctx.enter_context(tc.tile_pool(name="pads", bufs=2))
            qkp = ctx.enter_context(tc.tile_pool(name="qkp", bufs=3))
            smallp = ctx.enter_context(tc.tile_pool(name="smallp", bufs=2))

            # --- weights to SBUF ---
            wq_sb = wpool.tile([128, 6 * 256], E4, tag="wq")
            wk_sb = wpool.tile([128, 6 * 256], E4, tag="wk")
            v9_sb = wpool.tile([128, 2 * 576], E4, tag="v9")
            wp_sb = wpool.tile([128, 512], E4, tag="wp")
            wf_sb = wpool.tile([128, 512], BF16, tag="wf")
            w2t_sb = wpool.tile([128, 6 * 256], E5, tag="w2t")
            ident_sb = wpool.tile([128, 128], BF16, tag="ident")
            tcol = [wpool.tile([128, 1], F32, tag=f"tc{i}", name=f"tcol{i}")
                    for i in range(2)]
            nc.scalar.dma_start(out=wq_sb, in_=wq_d[:])
            nc.gpsimd.dma_start(out=wk_sb, in_=wk_d[:])
            nc.sync.dma_start(out=v9_sb, in_=v9_d[:])
            nc.sync.dma_start(out=wp_sb, in_=wp_d[:])
            nc.sync.dma_start(out=wf_sb, in_=wf_d[:])
            nc.sync.dma_start(out=tcol[0], in_=tc0_d[:])
            nc.sync.dma_start(out=tcol[1], in_=tc1_d[:])
            nc.sync.dma_start(out=ident_sb, in_=id_d[:])

            # --- pass-1 psums (qk pool created last, closes first: LIFO) ---
            p1stack = ctx.enter_context(contextlib.ExitStack())
            qkstack = ctx.enter_context(contextlib.ExitStack())
            ps_acc = p1stack.enter_context(
                tc.tile_pool(name="ps_acc", bufs=1, space="PSUM"))
            ps_qk = qkstack.enter_context(
                tc.tile_pool(name="ps_qk", bufs=3, space="PSUM"))
            acc1 = ps_acc.tile([128, 512], F32, tag="acc1")
            acc2 = ps_acc.tile([128, 256], F32, tag="acc2")
            par_all = acc1[:, 0:256]
            pgq = acc1[:, 256:512]
            pgk = acc2

            def build_pads(pool, dab, dad, name, band, dt8):
                """Load the [128, FA] padded AB|AD tile for one band: two
                contiguous DMAs from the host-baked shifted canvases."""
                P = pool.tile([128, FA], dt8, tag=f"pad{name}")
                o = (HB * band) * PW
                ap = P[:]
                nc.sync.dma_start(
                    out=ap[:, 0:TWA], in_=dab[:, o: o + TWA])
                nc.gpsimd.dma_start(
                    out=ap[:, ADB: ADB + TWA], in_=dad[:, o: o + TWA])
                return P

            def conv_drs(P, w_sb, pt, base, start_tag):
                """3 DR matmuls accumulating one row's conv into pt [128,256]."""
                ap = P[:]
                w3 = w_sb[:].rearrange("p (s n) -> p s n", n=256)
                pairs = [
                    (base, P0D, 0),
                    (base + 2 * PW, P1D, 2),
                    (ADB + base + 2, P2D, 4),
                ]
                for j, (X, D_, ws) in enumerate(pairs):
                    nc.tensor.matmul(
                        pt,
                        lhsT=_pair_ap(ap, X, D_, 128),
                        rhs=w3[:, ws: ws + 2, :],
                        start=(j == 0),
                        stop=(j == 2),
                        perf_mode=DR,
                    )

            # ================= pass 1: q/k convs + grams =================
            pend_gram = None  # (cat tile, first, last)

            def emit_grams(cat, first, last):
                cat3 = cat[:].rearrange("p (two c) -> p two c", two=2)
                for mb in range(2):
                    qsl = cat3[:, :, 256 + 128 * mb: 256 + 128 * mb + 128]
                    ksl = cat3[:, :, 128 * mb: 128 * mb + 128]
                    nc.tensor.matmul(
                        par_all[:, bass.ds(mb * 128, 128)],
                        lhsT=qsl, rhs=ksl, start=first, stop=last,
                        perf_mode=DR, skip_group_check=True)
                    nc.tensor.matmul(
                        pgq[:, bass.ds(mb * 128, 128)],
                        lhsT=qsl, rhs=qsl, start=first, stop=last,
                        perf_mode=DR, skip_group_check=True)
                    nc.tensor.matmul(
                        pgk[:, bass.ds(mb * 128, 128)],
                        lhsT=ksl, rhs=ksl, start=first, stop=last,
                        perf_mode=DR, skip_group_check=True)

            for b in range(NB):
                xP = build_pads(pads, pad_d["xab"], pad_d["xad"], "x", b, E4)
                yP = build_pads(pads, pad_d["yab"], pad_d["yad"], "y", b, E4)
                for hl2 in range(HB // 2):
                    pqk = ps_qk.tile([128, 1024], F32, tag="pqk")
                    for half in range(2):  # even/odd row of the pair
                        base = (2 * hl2 + half) * PW
                        o = 512 * half
                        conv_drs(yP, wk_sb, pqk[:, o: o + 256], base, "k")
                        conv_drs(xP, wq_sb, pqk[:, o + 256: o + 512], base, "q")
                    cat = qkp.tile([128, 1024], E4, tag="cat")
                    nc.scalar.copy(cat[:, 0:512], pqk[:, 0:512])
                    nc.vector.tensor_copy(cat[:, 512:1024], pqk[:, 512:1024])
                    if pend_gram is not None:
                        emit_grams(*pend_gram)
                    pg = 8 * b + hl2
                    pend_gram = (cat, pg == 0, pg == 63)
            emit_grams(*pend_gram)
            qkstack.close()
            # prefetch z band-0 pads early (DMA only; overlaps pass-1 tail)
            zP0 = build_pads(pads, pad_d["zab"], pad_d["zad"], "z", 0, E5)

            # ================= softmax on per-head [32,32] =================
            ar_sb = [smallp.tile([128, 128], F32, tag=f"arsb{mb}",
                                 name=f"arsb{mb}") for mb in range(2)]
            nc.scalar.copy(ar_sb[0], par_all[:, 0:128])
            nc.scalar.copy(ar_sb[1], par_all[:, 128:256])
            bd8 = [smallp.tile([128, 128], E4, tag=f"bd{mb}",
                               name=f"bdiag{mb}") for mb in range(2)]
            for mb in range(2):
                scr = smallp.tile([128, 128], F32, tag="scr")
                rnq_c = smallp.tile([128, 1], F32, tag="rnq")
                rnk_c = smallp.tile([128, 1], F32, tag="rnk")
                for g_ps, dst in ((pgq, rnq_c), (pgk, rnk_c)):
                    ssum = smallp.tile([128, 1], F32, tag="ssum")
                    nc.vector.tensor_mul(
                        scr, g_ps[:, bass.ds(mb * 128, 128)], ident_sb)
                    nc.vector.reduce_sum(
                        out=ssum, in_=scr, axis=mybir.AxisListType.X)
                    nc.scalar.sqrt(ssum, ssum)
                    nc.vector.tensor_scalar_max(ssum, ssum, 1e-12)
                    nc.vector.reciprocal(dst, ssum)
                rnqt = smallp.tile([128, 1], F32, tag="rnqt")
                nc.vector.tensor_mul(rnqt, rnq_c, tcol[mb])

                hd = smallp.tile([128, 32], F32, tag="hd")
                for i in range(4):
                    nc.vector.tensor_copy(
                        hd[32 * i: 32 * (i + 1), :],
                        ar_sb[mb][32 * i: 32 * (i + 1), bass.ds(32 * i, 32)],
                    )
                hds = smallp.tile([128, 32], F32, tag="hds")
                nc.scalar.activation(
                    hds, hd, mybir.ActivationFunctionType.Copy,
                    bias=0.0, scale=rnqt)
                hdT = smallp.tile([128, 32], F32, tag="hdT")
                nc.vector.transpose(hdT, hds)
                hdTs = smallp.tile([128, 32], F32, tag="hdTs")
                nc.scalar.activation(
                    hdTs, hdT, mybir.ActivationFunctionType.Copy,
                    bias=0.0, scale=rnk_c)
                hd3 = smallp.tile([128, 32], F32, tag="hd3")
                nc.vector.transpose(hd3, hdTs)
                nmx = smallp.tile([128, 1], F32, tag="nmx")
                nc.vector.reduce_max(
                    out=nmx, in_=hd3, axis=mybir.AxisListType.X, negate=True)
                ex = smallp.tile([128, 32], F32, tag="ex")
                nc.scalar.activation(
                    ex, hd3, mybir.ActivationFunctionType.Exp,
                    bias=nmx, scale=1.0)
                sm = smallp.tile([128, 1], F32, tag="sm")
                nc.vector.reduce_sum(out=sm, in_=ex, axis=mybir.AxisListType.X)
                rsm = smallp.tile([128, 1], F32, tag="rsm")
                nc.vector.reciprocal(rsm, sm)
                rsm32 = smallp.tile([128, 1], F32, tag="rsm32")
                nc.scalar.activation(
                    rsm32, rsm, mybir.ActivationFunctionType.Copy,
                    bias=0.0, scale=AT_SCALE)
                Pt = smallp.tile([128, 32], F32, tag="Pt")
                nc.scalar.activation(
                    Pt, ex, mybir.ActivationFunctionType.Copy,
                    bias=0.0, scale=rsm32)
                PtT = smallp.tile([128, 32], F32, tag="PtT")
                nc.vector.transpose(PtT, Pt)
                nc.gpsimd.memset(bd8[mb], 0.0)
                for i in range(4):
                    nc.vector.tensor_copy(
                        bd8[mb][32 * i: 32 * (i + 1), bass.ds(32 * i, 32)],
                        PtT[32 * i: 32 * (i + 1), :],
                    )

            # ================= W2T build =================
            p1stack.close()

            # --- phase-2 pools + prefill: the fus matmuls precede the
            # W-build matmuls in the PE queue, filling the softmax bubble ---
            p2p = ctx.enter_context(tc.tile_pool(name="p2p", bufs=4))
            ps_o = ctx.enter_context(
                tc.tile_pool(name="ps_o", bufs=4, space="PSUM"))

            def p2_fus(g):
                n0 = 512 * g
                xy_t = p2p.tile([128, 512], BF16, tag="xy")
                z_t = p2p.tile([64, 512], BF16, tag="zt")
                nc.gpsimd.dma_start(out=xy_t[0:64, :],
                                    in_=xd[:, bass.ds(n0, 512)])
                nc.gpsimd.dma_start(out=xy_t[64:128, :],
                                    in_=yd[:, bass.ds(n0, 512)])
                nc.gpsimd.dma_start(out=z_t, in_=zd[:, bass.ds(n0, 512)])
                pos = []
                for mb in range(2):
                    po = ps_o.tile([128, 512], F32, tag="po")
                    nc.tensor.matmul(
                        po, lhsT=wf_sb[:, bass.ds(mb * 128, 128)],
                        rhs=xy_t, start=True, stop=False)
                    nc.tensor.matmul(
                        po, lhsT=wf_sb[0:64, bass.ds(256 + mb * 128, 128)],
                        rhs=z_t, start=False, stop=False)
                    pos.append(po)
                return pos

            prefill = {g: p2_fus(g) for g in range(2)}

            wbstack = ctx.enter_context(contextlib.ExitStack())
            ps_w1 = wbstack.enter_context(
                tc.tile_pool(name="ps_w1", bufs=1, space="PSUM"))
            ps_w2 = wbstack.enter_context(
                tc.tile_pool(name="ps_w2", bufs=2, space="PSUM"))
            w1_sb = smallp.tile([128, 2 * 576], E4, tag="w1sb", name="w1sb")
            for mb in range(2):
                w1ps = ps_w1.tile([128, 576], F32, tag="w1ps")
                vsl = v9_sb[:, 576 * mb: 576 * mb + 576]
                nc.tensor.matmul(w1ps[:, 0:512], lhsT=bd8[mb],
                                 rhs=vsl[:, 0:512], start=True, stop=True)
                nc.tensor.matmul(w1ps[:, 512:576], lhsT=bd8[mb],
                                 rhs=vsl[:, 512:576], start=True, stop=True)
                (nc.scalar.copy if mb == 0 else nc.vector.tensor_copy)(
                    w1_sb[:, 576 * mb: 576 * mb + 576], w1ps)
            w13 = w1_sb[:].rearrange("p (two k) -> p two k", two=2)
            wp3 = wp_sb[:].rearrange("p (two n) -> p two n", two=2)
            # slot 4 of w2t is zeros; k-chunk 4 goes to slot 5
            nc.gpsimd.memset(w2t_sb[:, 4 * 256: 5 * 256], 0.0)
            nc.gpsimd.memset(w2t_sb[64:128, 5 * 256: 6 * 256], 0.0)
            for j in range(5):
                kw = 128 if j < 4 else 64
                w2ps = ps_w2.tile([128, 256], F32, tag="w2ps")
                nc.tensor.matmul(
                    w2ps[0:kw, :],
                    lhsT=w13[:, :, 128 * j: 128 * j + kw],
                    rhs=wp3, start=True, stop=True, perf_mode=DR)
                slot = j if j < 4 else 5
                nc.scalar.activation(
                    w2t_sb[0:kw, slot * 256: (slot + 1) * 256], w2ps[0:kw, :],
                    mybir.ActivationFunctionType.Copy,
                    bias=0.0, scale=float(W2_COPY_SCALE))

            # ================= phase 2: fus + W2T@z9 =================
            wbstack.close()
            w2t3 = w2t_sb[:].rearrange("p (s n) -> p s n", n=256)

            def p2_attn_out(g, zap, pos):
                n0 = 512 * g
                cc = g % 4
                for mb in range(2):
                    po = pos[mb]
                    for r in range(4):
                        base = (4 * cc + r) * PW
                        pairs = [
                            (base, P0D, 0),
                            (base + 2 * PW, P1D, 2),
                            (ADB + base + 2, P2D, 4),
                        ]
                        for j, (X, D_, ws) in enumerate(pairs):
                            nc.tensor.matmul(
                                po[:, 128 * r: 128 * r + 128],
                                lhsT=w2t3[:, ws: ws + 2,
                                          128 * mb: 128 * mb + 128],
                                rhs=_pair_ap(zap, X, D_, 128),
                                start=False,
                                stop=(r == 3 and j == 2),
                                perf_mode=DR,
                                skip_group_check=True)
                    o_t = p2p.tile([128, 512], BF16, tag=f"ot{mb}",
                                   name="o_t")
                    if mb == 0:
                        nc.scalar.copy(o_t, po)
                    else:
                        nc.vector.tensor_copy(o_t, po)
                    (nc.sync if mb == 0 else nc.gpsimd).dma_start(
                        out=od[bass.ds(mb * 128, 128), bass.ds(n0, 512)],
                        in_=o_t)

            for bz in range(NB):
                zP = zP0 if bz == 0 else build_pads(
                    pads, pad_d["zab"], pad_d["zad"], "z", bz, E5)
                zap = zP[:]
                for cc in range(4):
                    g = 4 * bz + cc
                    pos = prefill.pop(g, None) or p2_fus(g)
                    p2_attn_out(g, zap, pos)

    _split_excess_waits(nc)
    return nc


def _prep_weights(inputs):
    wq = _merge_w(np.asarray(inputs["Wq"], np.float32),
                  np.asarray(inputs["Wq_dw"], np.float32), QK_SCALE)
    wk = _merge_w(np.asarray(inputs["Wk"], np.float32),
                  np.asarray(inputs["Wk_dw"], np.float32), QK_SCALE)
    v9 = _merge_v9(np.asarray(inputs["Wv"], np.float32),
                   np.asarray(inputs["Wv_dw"], np.float32), V9_SCALE)

    wproj = np.asarray(inputs["Wproj"], np.float32)[:, :, 0, 0]  # [256,256]
    # WprojN [c, o] mb tiles side by side: [128, 512]
    wprojN = np.zeros((128, 512), np.float32)
    wprojN[:, 0:256] = wproj[:, 0:128].T * PR_SCALE
    wprojN[:, 256:512] = wproj[:, 128:256].T * PR_SCALE

    wfus = np.asarray(inputs["Wfus"], np.float32)[:, :, 0, 0]  # [256, 192]
    wfusT = np.zeros((128, 512), np.float32)
    wfusT[:, 0:256] = wfus[:, 0:128].T          # x,y rows
    wfusT[0:64, 256:512] = wfus[:, 128:192].T   # z rows

    temp = np.asarray(inputs["temperature"], np.float32).reshape(HEADS)
    tfull = np.repeat(temp, 32).astype(np.float32)
    temp_cols = [tfull[0:128].reshape(128, 1), tfull[128:256].reshape(128, 1)]
    return wq, wk, v9, wprojN, wfusT, temp_cols


def _canvases(img, np8):
    """img [64, 128, 128] fp32 -> (ab, ad) [128, 130*PW] canvases in np8:
    parts 0:64 = A (padded image at pitch PW), 64:128 = B (A<<1col) / D
    (A<<1row)."""
    LC = 130 * PW
    A = np.zeros((64, 130, PW), np.float32)
    A[:, 1:129, 1:129] = img
    Af = A.reshape(64, LC)
    ext = np.zeros((64, LC + PW + 8), np.float32)
    ext[:, :LC] = Af
    ab = np.zeros((128, LC), np.float32)
    ab[0:64] = Af
    ab[64:128] = ext[:, 1: LC + 1]
    ad = np.zeros((128, LC), np.float32)
    ad[0:64] = Af
    ad[64:128] = ext[:, PW: LC + PW]
    return ab.astype(np8), ad.astype(np8)


def kernel(**inputs):
    x = np.asarray(inputs["x"], np.float32)
    y = np.asarray(inputs["y"], np.float32)
    z = np.asarray(inputs["z"], np.float32)
    B = x.shape[0]
    assert B == 8

    nc = _build_nc(*_prep_weights(inputs))

    in_maps = []
    for i in range(B):
        xi = x[i].reshape(C, N)
        yi = y[i].reshape(C, N)
        zi = z[i].reshape(C, N)
        xab, xad = _canvases(x[i], NP_E4)
        yab, yad = _canvases(y[i], NP_E4)
        zab, zad = _canvases(z[i] * Z8_SCALE, NP_E5)
        in_maps.append({
            "x": _bf(xi),
            "y": _bf(yi),
            "z": _bf(zi),
            "xab": xab, "xad": xad,
            "yab": yab, "yad": yad,
            "zab": zab, "zad": zad,
        })
    res = run_bass_kernel_spmd(nc, in_maps, list(range(8)))
    out = np.stack(
        [np.asarray(res.results[i]["out"]).astype(np.float32).reshape(DIM, H, W)
         for i in range(B)]
    )
    return out


# revision 33
# speedup vs baseline: 2.7598x; 1.1439x over previous
"""Trainium2 Bass kernel for nn_LHFA_76278619177511.

Fused transposed-attention block (LHFA):
  q = dwconv3(conv1x1(x, Wq), Wq_dw)   (k from y, v from z)
  attn = softmax(l2norm(q) @ l2norm(k)^T * temp)   per-head [32,32]
  out = Wproj @ (attn @ v) + Wfus @ [x;y;z]

V2 strategy (pure DP over batch B=8 on 8 cores), key ideas:
  - fp8 DoubleRow matmuls (0.5 cycles/row, 2 K-tiles per instruction):
    the merged 1x1+dw conv (K=576) runs in 3 DR matmuls instead of 5
    bf16 passes. Pad row pitch = 144 so every DR pair stride/offset is
    16-aligned (dual-fp8 Ldweights ISA restriction).
  - q/k path entirely in fp8-e4m3 with weights pre-scaled x128; the
    scale cancels exactly in the L2 normalization.
  - The whole v path is folded into weights: W2T = (Wproj@attn@V9)^T is
    built on-device with tiny matmuls after softmax, then the attention
    output term becomes a single fused conv W2T^T @ z9 in fp8-e5m2
    accumulated INTO THE SAME PSUM as the bf16 fus conv (scales cancel:
    W2T stored x2^8, z fed x2^-8). No v slab, no attn@v, no proj.
  - Grams/sq-norms via DR on row pairs (64 pairs over the image).
  - attn output term is ~500x smaller than the fus term, so fp8 noise
    in it is invisible at the 2e-2 gate; fus stays bf16.
"""

import numpy as np
import ml_dtypes

import bass_rust
import concourse.bass as bass
import concourse.mybir as mybir
from concourse import tile as tile_mod
from concourse.tile import TileContext
from concourse.vector_clock import ScopedClock
from concourse.bass_utils import run_bass_kernel_spmd

BF16 = mybir.dt.bfloat16
F32 = mybir.dt.float32
E4 = mybir.dt.float8e4
E5 = mybir.dt.float8e5
NP_E4 = ml_dtypes.float8_e4m3
NP_E5 = ml_dtypes.float8_e5m2
NP_BF = ml_dtypes.bfloat16
DR = mybir.MatmulPerfMode.DoubleRow

C = 64          # input channels
DIM = 256       # q/k/v channels
HEADS = 8
H = W = 128
N = H * W       # 16384
PW = 144        # padded row pitch (16-aligned; image cols at [1,129))
HB = 16         # band height
NB = H // HB    # 8 bands
NRT = HB + 2    # rows per band tile
TWA = NRT * PW  # 2592: region width (AB / AD each)
ADB = 2606      # AD region base col (== 14 mod 16 so P1 stride is 16-mult)
FA = 7168       # pad tile alloc width (slice-bound slack)

# DR pair definitions: (X0(base), delta). base = hl*PW.
#   P0: (S0 @ AB+base,        S1 @ AB+base+PW)        taps rows -1, 0
#   P1: (S2 @ AB+base+2PW,    S3 @ AD+base+2)         taps row +1, col +1
#   P2: (dummy @ AD+base+2,   S4 @ AD+base+2PW+2)     tap (1,1) (A only)
P0D = PW
P1D = ADB + 2 - 2 * PW   # 2320
P2D = 2 * PW             # 288
assert P1D % 16 == 0 and ADB % 16 == 14
# weight slots [6 x 256]: 0=S0 1=S1 2=S2 3=S3 4=ZERO 5=S4
# taps (dy,dx) per slot/half for weight merging (half 0 = A rows,
# half 1 = B rows (slots 0-2, B = A<<1col) or D rows (slot 3, D = A<<1row))
SLOT_TAPS = [
    [(-1, -1), (-1, 0)],
    [(0, -1), (0, 0)],
    [(1, -1), (1, 0)],
    [(-1, 1), (0, 1)],
    [],
    [(1, 1)],
]

QK_SCALE = 128.0      # q/k conv weight prescale (cancels in l2norm)
V9_SCALE = 2.0 ** 12  # V9 prescale into e4m3
PR_SCALE = 2.0 ** 6   # WprojN prescale into e4m3
AT_SCALE = 32.0       # attn (Pt) prescale into e4m3
W2_OUT_SCALE = 2.0 ** 8   # W2T stored scale (z fed at 2^-8)
Z8_SCALE = 2.0 ** -8
# W2 psum carries AT*V9*PR = 2^23; store at 2^8 -> copy scale 2^-15
W2_COPY_SCALE = W2_OUT_SCALE / (AT_SCALE * V9_SCALE * PR_SCALE)

_PATCHED = False


def _patch_tile_drain():
    """This walrus build rejects >1 sem wait on a CTRL (Drain) instruction;
    split the TileContext tail-drain waits onto individual nops."""
    global _PATCHED
    if _PATCHED:
        return
    _PATCHED = True

    def _drain_and_barrier(self, tick_clock, wait_clock):
        nc = self.nc
        drain_inst = nc.sync.drain()
        wait_clock.add_sem_waits(
            drain_inst.ins, ScopedClock({None: tick_clock.global_clock})
        )
        si = drain_inst.ins.sync_info
        waits = list(si.on_wait or [])
        if len(waits) > 1:
            si.on_wait = waits[:1]
            for w in waits[1:]:
                nop = nc.sync.nop(nofuse=True, hint="split_wait")
                nop.ins.sync_info = bass_rust.SyncInfo(on_wait=[w], on_update=[])
        nc.all_engine_barrier()
        assert self.sems is not None
        popped = nc._tile_sem_poison_stack.pop()
        assert popped is self._sem_poison
        nc.clear_and_free_semaphores(list(self.sems.allocated().values()))
        nc.all_engine_barrier()

    tile_mod.TileContext._drain_and_barrier = _drain_and_barrier
    try:
        from concourse import tile_utils
        tile_utils.max_sbuf_usage = 208 * 1024
    except Exception:
        pass


def _split_excess_waits(nc, max_waits=1):
    """This walrus build caps sem waits per instruction encoding; hoist
    excess waits onto preceding same-engine NoOps (queues are in-order,
    so a wait satisfied on an earlier instruction orders the later one)."""
    import bass_rust as _br

    ctr = [0]
    for f in nc.m.functions:
        for blk in f.blocks:
            out = []
            for inst in blk.instructions:
                si = inst.sync_info
                waits = list(si.on_wait) if (si and si.on_wait) else []
                if len(waits) > max_waits:
                    keep = waits[:max_waits]
                    extra = waits[max_waits:]
                    si.on_wait = keep
                    for w in extra:
                        ctr[0] += 1
                        nop = _br.InstNoOp(name=f"wsplit-{ctr[0]}", ins=[], outs=[])
                        nop.engine = inst.engine
                        nop.sync_info = _br.SyncInfo(on_wait=[w], on_update=[])
                        try:
                            nc.register_instruction(nop, overwrite=True)
                        except Exception:
                            pass
                        out.append(nop)
                out.append(inst)
            blk.instructions[:] = out


def _merge_w(W1, Wdw, scale):
    """-> [128, 6, 256] float32 merged conv weights in slot layout.
    [p=(half,chan), slot, outch] = dw[out, tap(slot, half)] * W1[out, chan]."""
    out = np.zeros((128, 6, 256), np.float32)
    W1 = W1[:, :, 0, 0]  # [256, 64]
    for s, taps in enumerate(SLOT_TAPS):
        for half, (dy, dx) in enumerate(taps):
            hh = half if s != 5 else 0
            out[hh * 64: (hh + 1) * 64, s, :] = (
                Wdw[:, 0, 1 + dy, 1 + dx][:, None] * W1
            ).T * scale
    return out


def _merge_v9(W1, Wdw, scale):
    """-> [256, 576] float32: V9[d, s*128 + half*64 + c] for slots 0-3 +
    slot5(A half) packed as k-chunk order s in 0..4 (chunk4 = slot5 taps)."""
    out = np.zeros((256, 640), np.float32)
    W1 = W1[:, :, 0, 0]  # [256, 64]
    for s, taps in enumerate(SLOT_TAPS):
        if s == 4:
            continue
        kc = s if s < 4 else 4
        for half, (dy, dx) in enumerate(taps):
            hh = half if s != 5 else 0
            out[:, kc * 128 + hh * 64: kc * 128 + (hh + 1) * 64] = (
                Wdw[:, 0, 1 + dy, 1 + dx][:, None] * W1
            ) * scale
    return out[:, :576]


def _bf(a):
    return np.ascontiguousarray(a).astype(NP_BF)


def _pair_ap(tile_ap, X, delta, width):
    """[128, 2, width] AP: members at cols X and X+delta."""
    return tile_ap[:, X: X + 2 * delta].rearrange(
        "p (two m) -> p two m", two=2
    )[:, :, 0:width]


def _build_nc(wq, wk, v9, wprojN, wfusT, temp_cols):
    """Build the Bass module. Weight arrays pre-merged/scaled fp32."""
    _patch_tile_drain()
    nc = bass.Bass()

    # xy interleaved for the fus loads: parts 0:64 = x, 64:128 = y
    xyd = nc.declare_dram_parameter("xy", [128, N], BF16, isOutput=False)
    zd = nc.declare_dram_parameter("z", [C, N], BF16, isOutput=False)
    # pre-padded, pre-shifted canvases [128, 2*LC]: cols 0:LC = AB
    # (parts 0:64 = A padded image, 64:128 = B = A<<1col), cols LC:2LC =
    # AD (A | D = A<<1row)
    LC = 130 * PW
    pad_d = {}
    for nm, dt8 in (("xc", E4), ("yc", E4), ("zc", E5)):
        pad_d[nm] = nc.declare_dram_parameter(
            nm, [128, 2 * LC], dt8, isOutput=False)
    od = nc.declare_dram_parameter("out", [DIM, N], BF16, isOutput=True)

    wq_d = nc.inline_tensor(
        np.ascontiguousarray(wq.reshape(128, 6 * 256)).astype(NP_E4), name="wq9")
    wk_d = nc.inline_tensor(
        np.ascontiguousarray(wk.reshape(128, 6 * 256)).astype(NP_E4), name="wk9")
    # V9 [256,576] -> 2 mb tiles side by side [128, 1152]
    v9_2 = np.concatenate([v9[0:128], v9[128:256]], axis=1)
    v9_d = nc.inline_tensor(np.ascontiguousarray(v9_2).astype(NP_E4), name="v9")
    wp_d = nc.inline_tensor(
        np.ascontiguousarray(wprojN).astype(NP_E4), name="wprojN")  # [128, 512]
    wf_d = nc.inline_tensor(_bf(wfusT), name="wfusT")               # [128, 512]
    tc0_d = nc.inline_tensor(np.ascontiguousarray(temp_cols[0]), name="tcol0")
    tc1_d = nc.inline_tensor(np.ascontiguousarray(temp_cols[1]), name="tcol1")
    id_d = nc.inline_tensor(np.eye(128, dtype=NP_BF), name="ident")

    with TileContext(nc) as tc:
        import contextlib

        with contextlib.ExitStack() as ctx:
            wpool = ctx.enter_context(tc.tile_pool(name="wpool", bufs=1))
            pads = ctx.enter_context(tc.tile_pool(name="pads", bufs=3))
            qkp = ctx.enter_context(tc.tile_pool(name="qkp", bufs=3))
            smallp = ctx.enter_context(tc.tile_pool(name="smallp", bufs=2))

            # --- weights to SBUF ---
            wq_sb = wpool.tile([128, 6 * 256], E4, tag="wq")
            wk_sb = wpool.tile([128, 6 * 256], E4, tag="wk")
            v9_sb = wpool.tile([128, 2 * 576], E4, tag="v9")
            wp_sb = wpool.tile([128, 512], E4, tag="wp")
            wf_sb = wpool.tile([128, 512], BF16, tag="wf")
            # w2t as 3 separate pair tiles so phase-2 DRs only wait on the
            # slots they read
            w2t_t = [wpool.tile([128, 512], E5, tag=f"w2t{k}",
                                name=f"w2t{k}") for k in range(3)]
            ident_sb = wpool.tile([128, 128], BF16, tag="ident")
            tcol = [wpool.tile([128, 1], F32, tag=f"tc{i}", name=f"tcol{i}")
                    for i in range(2)]

            # --- pass-1 psums (qk pool created last, closes first: LIFO) ---
            p1stack = ctx.enter_context(contextlib.ExitStack())
            qkstack = ctx.enter_context(contextlib.ExitStack())
            ps_acc = p1stack.enter_context(
                tc.tile_pool(name="ps_acc", bufs=1, space="PSUM"))
            ps_qk = qkstack.enter_context(
                tc.tile_pool(name="ps_qk", bufs=3, space="PSUM"))
            acc1 = ps_acc.tile([128, 512], F32, tag="acc1")
            acc2 = ps_acc.tile([128, 256], F32, tag="acc2")
            par_all = acc1[:, 0:256]
            pgq = acc1[:, 256:512]
            pgk = acc2

            def build_pads(pool, dc, name, band, dt8):
                """Load the [128, FA] padded AB|AD tile for one band in ONE
                two-range DMA from the host-baked shifted canvas."""
                P = pool.tile([128, FA], dt8, tag=f"pad{name}")
                o = (HB * band) * PW
                src = dc[:].rearrange(
                    "p (two l) -> p two l", l=LC)[:, :, o: o + TWA]
                dst = P[:][:, 0: 2 * ADB].rearrange(
                    "p (two w) -> p two w", two=2)[:, :, 0:TWA]
                nc.sync.dma_start(out=dst, in_=src)
                return P

            # band-0 pads first so the first conv's data leads the DMA
            # device queue; weights interleave behind them
            xP0 = build_pads(pads, pad_d["xc"], "x", 0, E4)
            nc.scalar.dma_start(out=wq_sb, in_=wq_d[:])
            yP0 = build_pads(pads, pad_d["yc"], "y", 0, E4)
            nc.scalar.dma_start(out=wk_sb, in_=wk_d[:])
            nc.gpsimd.dma_start(out=v9_sb, in_=v9_d[:])
            nc.gpsimd.dma_start(out=wp_sb, in_=wp_d[:])
            nc.gpsimd.dma_start(out=wf_sb, in_=wf_d[:])
            nc.gpsimd.dma_start(out=tcol[0], in_=tc0_d[:])
            nc.gpsimd.dma_start(out=tcol[1], in_=tc1_d[:])
            nc.gpsimd.dma_start(out=ident_sb, in_=id_d[:])

            def conv_drs(P, w_sb, pt, base, start_tag):
                """3 DR matmuls accumulating one row's conv into pt [128,256]."""
                ap = P[:]
                w3 = w_sb[:].rearrange("p (s n) -> p s n", n=256)
                pairs = [
                    (base, P0D, 0),
                    (base + 2 * PW, P1D, 2),
                    (ADB + base + 2, P2D, 4),
                ]
                for j, (X, D_, ws) in enumerate(pairs):
                    nc.tensor.matmul(
                        pt,
                        lhsT=_pair_ap(ap, X, D_, 128),
                        rhs=w3[:, ws: ws + 2, :],
                        start=(j == 0),
                        stop=(j == 2),
                        perf_mode=DR,
                    )

            # ================= pass 1: q/k convs + grams =================
            pend_gram = None  # (cat tile, first, last)

            def emit_grams(cat, first, last):
                cat3 = cat[:].rearrange("p (two c) -> p two c", two=2)
                for mb in range(2):
                    qsl = cat3[:, :, 256 + 128 * mb: 256 + 128 * mb + 128]
                    ksl = cat3[:, :, 128 * mb: 128 * mb + 128]
                    nc.tensor.matmul(
                        par_all[:, bass.ds(mb * 128, 128)],
                        lhsT=qsl, rhs=ksl, start=first, stop=last,
                        perf_mode=DR, skip_group_check=True)
                    nc.tensor.matmul(
                        pgq[:, bass.ds(mb * 128, 128)],
                        lhsT=qsl, rhs=qsl, start=first, stop=last,
                        perf_mode=DR, skip_group_check=True)
                    nc.tensor.matmul(
                        pgk[:, bass.ds(mb * 128, 128)],
                        lhsT=ksl, rhs=ksl, start=first, stop=last,
                        perf_mode=DR, skip_group_check=True)

            # persistent fus-input slabs, preloaded during pass 1 (the DMA
            # device has slack there; phase 2 then only moves pads + output)
            fuspool = ctx.enter_context(tc.tile_pool(name="fusp", bufs=1))
            xy_slab = fuspool.tile([128, N], BF16, tag="xyslab", name="xyslab")
            z_slab = fuspool.tile([64, N], BF16, tag="zslab", name="zslab")

            for b in range(NB):
                xP = xP0 if b == 0 else build_pads(
                    pads, pad_d["xc"], "x", b, E4)
                yP = yP0 if b == 0 else build_pads(
                    pads, pad_d["yc"], "y", b, E4)
                n0 = b * (N // NB)
                nc.gpsimd.dma_start(
                    out=xy_slab[:, bass.ds(n0, N // NB)],
                    in_=xyd[:, bass.ds(n0, N // NB)])
                nc.gpsimd.dma_start(
                    out=z_slab[:, bass.ds(n0, N // NB)],
                    in_=zd[:, bass.ds(n0, N // NB)])
                for hl2 in range(HB // 2):
                    pqk = ps_qk.tile([128, 1024], F32, tag="pqk")
                    for half in range(2):  # even/odd row of the pair
                        base = (2 * hl2 + half) * PW
                        o = 512 * half
                        conv_drs(yP, wk_sb, pqk[:, o: o + 256], base, "k")
                        conv_drs(xP, wq_sb, pqk[:, o + 256: o + 512], base, "q")
                    cat = qkp.tile([128, 1024], E4, tag="cat")
                    nc.scalar.copy(cat[:, 0:512], pqk[:, 0:512])
                    nc.vector.tensor_copy(cat[:, 512:1024], pqk[:, 512:1024])
                    if pend_gram is not None:
                        emit_grams(*pend_gram)
                    pg = 8 * b + hl2
                    pend_gram = (cat, pg == 0, pg == 63)
            emit_grams(*pend_gram)
            qkstack.close()
            # prefetch z band-0 pads early (DMA only; overlaps pass-1 tail)
            zP0 = build_pads(pads, pad_d["zc"], "z", 0, E5)

            # --- phase-2 pools + prefill: fus matmuls run on the PE while
            # the softmax chain occupies ACT/DVE ---
            p2p = ctx.enter_context(tc.tile_pool(name="p2p", bufs=4))
            ps_o = ctx.enter_context(
                tc.tile_pool(name="ps_o", bufs=6, space="PSUM"))

            def p2_fus(g):
                n0 = 512 * g
                pos = []
                for mb in range(2):
                    po = ps_o.tile([128, 512], F32, tag="po")
                    nc.tensor.matmul(
                        po, lhsT=wf_sb[:, bass.ds(mb * 128, 128)],
                        rhs=xy_slab[:, bass.ds(n0, 512)],
                        start=True, stop=False)
                    nc.tensor.matmul(
                        po, lhsT=wf_sb[0:64, bass.ds(256 + mb * 128, 128)],
                        rhs=z_slab[:, bass.ds(n0, 512)],
                        start=False, stop=False)
                    pos.append(po)
                return pos

            # slot 4 (pair tile 2, first half) is zeros; k-chunk 4 -> slot 5
            nc.gpsimd.memset(w2t_t[2][:, 0:256], 0.0)
            nc.gpsimd.memset(w2t_t[2][64:128, 256:512], 0.0)
            prefill = {g: p2_fus(g) for g in range(3)}

            # ================= softmax on per-head [32,32] =================
            ar_sb = [smallp.tile([128, 128], F32, tag=f"arsb{mb}",
                                 name=f"arsb{mb}") for mb in range(2)]
            nc.scalar.copy(ar_sb[0], par_all[:, 0:128])
            nc.scalar.copy(ar_sb[1], par_all[:, 128:256])
            bd8 = [smallp.tile([128, 128], E4, tag=f"bd{mb}",
                               name=f"bdiag{mb}") for mb in range(2)]
            for mb in range(2):
                scr = smallp.tile([128, 128], F32, tag="scr")
                rnq_c = smallp.tile([128, 1], F32, tag="rnq")
                rnk_c = smallp.tile([128, 1], F32, tag="rnk")
                for g_ps, dst in ((pgq, rnq_c), (pgk, rnk_c)):
                    ssum = smallp.tile([128, 1], F32, tag="ssum")
                    nc.vector.tensor_mul(
                        scr, g_ps[:, bass.ds(mb * 128, 128)], ident_sb)
                    nc.vector.reduce_sum(
                        out=ssum, in_=scr, axis=mybir.AxisListType.X)
                    nc.scalar.sqrt(ssum, ssum)
                    nc.vector.tensor_scalar_max(ssum, ssum, 1e-12)
                    nc.vector.reciprocal(dst, ssum)
                rnqt = smallp.tile([128, 1], F32, tag="rnqt")
                nc.vector.tensor_mul(rnqt, rnq_c, tcol[mb])

                hd = smallp.tile([128, 32], F32, tag="hd")
                for i in range(4):
                    nc.vector.tensor_copy(
                        hd[32 * i: 32 * (i + 1), :],
                        ar_sb[mb][32 * i: 32 * (i + 1), bass.ds(32 * i, 32)],
                    )
                hds = smallp.tile([128, 32], F32, tag="hds")
                nc.scalar.activation(
                    hds, hd, mybir.ActivationFunctionType.Copy,
                    bias=0.0, scale=rnqt)
                hdT = smallp.tile([128, 32], F32, tag="hdT")
                nc.vector.transpose(hdT, hds)
                hdTs = smallp.tile([128, 32], F32, tag="hdTs")
                nc.scalar.activation(
                    hdTs, hdT, mybir.ActivationFunctionType.Copy,
                    bias=0.0, scale=rnk_c)
                hd3 = smallp.tile([128, 32], F32, tag="hd3")
                nc.vector.transpose(hd3, hdTs)
                nmx = smallp.tile([128, 1], F32, tag="nmx")
                nc.vector.reduce_max(
                    out=nmx, in_=hd3, axis=mybir.AxisListType.X, negate=True)
                ex = smallp.tile([128, 32], F32, tag="ex")
                nc.scalar.activation(
                    ex, hd3, mybir.ActivationFunctionType.Exp,
                    bias=nmx, scale=1.0)
                sm = smallp.tile([128, 1], F32, tag="sm")
                nc.vector.reduce_sum(out=sm, in_=ex, axis=mybir.AxisListType.X)
                rsm = smallp.tile([128, 1], F32, tag="rsm")
                nc.vector.reciprocal(rsm, sm)
                rsm32 = smallp.tile([128, 1], F32, tag="rsm32")
                nc.scalar.activation(
                    rsm32, rsm, mybir.ActivationFunctionType.Copy,
                    bias=0.0, scale=AT_SCALE)
                Pt = smallp.tile([128, 32], F32, tag="Pt")
                nc.scalar.activation(
                    Pt, ex, mybir.ActivationFunctionType.Copy,
                    bias=0.0, scale=rsm32)
                PtT = smallp.tile([128, 32], F32, tag="PtT")
                nc.vector.transpose(PtT, Pt)
                nc.gpsimd.memset(bd8[mb], 0.0)
                for i in range(4):
                    nc.vector.tensor_copy(
                        bd8[mb][32 * i: 32 * (i + 1), bass.ds(32 * i, 32)],
                        PtT[32 * i: 32 * (i + 1), :],
                    )

            # ================= W2T build (reuses acc psum banks) =======
            # W1 = (attn*32) @ V9 into the dead gram psums: acc1 holds
            # cols 0:512, acc2[:, 192:256] the 64-tail
            w1_sb = smallp.tile([128, 2 * 576], E4, tag="w1sb", name="w1sb")
            for mb in range(2):
                vsl = v9_sb[:, 576 * mb: 576 * mb + 576]
                nc.tensor.matmul(acc1, lhsT=bd8[mb],
                                 rhs=vsl[:, 0:512], start=True, stop=True,
                                 skip_group_check=True)
                nc.tensor.matmul(acc2[:, 192:256], lhsT=bd8[mb],
                                 rhs=vsl[:, 512:576], start=True, stop=True,
                                 skip_group_check=True)
                nc.scalar.copy(w1_sb[:, 576 * mb: 576 * mb + 512], acc1)
                nc.vector.tensor_copy(
                    w1_sb[:, 576 * mb + 512: 576 * mb + 576],
                    acc2[:, 192:256])
            w13 = w1_sb[:].rearrange("p (two k) -> p two k", two=2)
            wp3 = wp_sb[:].rearrange("p (two n) -> p two n", two=2)
            for j in range(5):
                kw = 128 if j < 4 else 64
                nc.tensor.matmul(
                    acc2[0:kw, :],
                    lhsT=w13[:, :, 128 * j: 128 * j + kw],
                    rhs=wp3, start=True, stop=True, perf_mode=DR,
                    skip_group_check=True)
                dstt = w2t_t[j // 2] if j < 4 else w2t_t[2]
                dsts = (j % 2) if j < 4 else 1
                nc.scalar.activation(
                    dstt[0:kw, dsts * 256: (dsts + 1) * 256], acc2[0:kw, :],
                    mybir.ActivationFunctionType.Copy,
                    bias=0.0, scale=float(W2_COPY_SCALE))

            # ================= phase 2: fus + W2T@z9 =================
            w2t3 = [t[:].rearrange("p (s n) -> p s n", n=256) for t in w2t_t]
            od3 = od[:].rearrange("(two p) c -> p two c", two=2)

            def p2_attn_out(g, zap, pos):
                n0 = 512 * g
                cc = g % 4
                o2 = p2p.tile([128, 1024], BF16, tag="o2", name="o2")
                for mb in range(2):
                    po = pos[mb]
                    for r in range(4):
                        base = (4 * cc + r) * PW
                        pairs = [
                            (base, P0D, 0),
                            (base + 2 * PW, P1D, 1),
                            (ADB + base + 2, P2D, 2),
                        ]
                        for j, (X, D_, wk_) in enumerate(pairs):
                            nc.tensor.matmul(
                                po[:, 128 * r: 128 * r + 128],
                                lhsT=w2t3[wk_][:, :,
                                               128 * mb: 128 * mb + 128],
                                rhs=_pair_ap(zap, X, D_, 128),
                                start=False,
                                stop=(r == 3 and j == 2),
                                perf_mode=DR,
                                skip_group_check=True)
                    if mb == 0:
                        nc.scalar.copy(o2[:, 0:512], po)
                    else:
                        nc.vector.tensor_copy(o2[:, 512:1024], po)
                # one DMA for both halves: dst rows (p, p+128), cols n0..+512
                dst = od3[:, :, bass.ds(n0, 512)]
                src = o2[:].rearrange("p (two c) -> p two c", two=2)
                nc.sync.dma_start(out=dst, in_=src)

            for bz in range(NB):
                zP = zP0 if bz == 0 else build_pads(
                    pads, pad_d["zc"], "z", bz, E5)
                zap = zP[:]
                for cc in range(4):
                    g = 4 * bz + cc
                    pos = prefill.pop(g, None) or p2_fus(g)
                    p2_attn_out(g, zap, pos)

    _split_excess_waits(nc)
    return nc


def _prep_weights(inputs):
    wq = _merge_w(np.asarray(inputs["Wq"], np.float32),
                  np.asarray(inputs["Wq_dw"], np.float32), QK_SCALE)
    wk = _merge_w(np.asarray(inputs["Wk"], np.float32),
                  np.asarray(inputs["Wk_dw"], np.float32), QK_SCALE)
    v9 = _merge_v9(np.asarray(inputs["Wv"], np.float32),
                   np.asarray(inputs["Wv_dw"], np.float32), V9_SCALE)

    wproj = np.asarray(inputs["Wproj"], np.float32)[:, :, 0, 0]  # [256,256]
    # WprojN [c, o] mb tiles side by side: [128, 512]
    wprojN = np.zeros((128, 512), np.float32)
    wprojN[:, 0:256] = wproj[:, 0:128].T * PR_SCALE
    wprojN[:, 256:512] = wproj[:, 128:256].T * PR_SCALE

    wfus = np.asarray(inputs["Wfus"], np.float32)[:, :, 0, 0]  # [256, 192]
    wfusT = np.zeros((128, 512), np.float32)
    wfusT[:, 0:256] = wfus[:, 0:128].T          # x,y rows
    wfusT[0:64, 256:512] = wfus[:, 128:192].T   # z rows

    temp = np.asarray(inputs["temperature"], np.float32).reshape(HEADS)
    tfull = np.repeat(temp, 32).astype(np.float32)
    temp_cols = [tfull[0:128].reshape(128, 1), tfull[128:256].reshape(128, 1)]
    return wq, wk, v9, wprojN, wfusT, temp_cols


def _canvas(img, np8):
    """img [64, 128, 128] fp32 -> [128, 2*130*PW] canvas in np8: cols
    [0, LC) = AB (parts 0:64 = A padded image at pitch PW, 64:128 =
    B = A<<1col), cols [LC, 2LC) = AD (A | D = A<<1row)."""
    LC = 130 * PW
    A = np.zeros((64, 130, PW), np.float32)
    A[:, 1:129, 1:129] = img
    Af = A.reshape(64, LC)
    ext = np.zeros((64, LC + PW + 8), np.float32)
    ext[:, :LC] = Af
    out = np.zeros((128, 2 * LC), np.float32)
    out[0:64, 0:LC] = Af
    out[64:128, 0:LC] = ext[:, 1: LC + 1]
    out[0:64, LC:] = Af
    out[64:128, LC:] = ext[:, PW: LC + PW]
    return out.astype(np8)


def kernel(**inputs):
    x = np.asarray(inputs["x"], np.float32)
    y = np.asarray(inputs["y"], np.float32)
    z = np.asarray(inputs["z"], np.float32)
    B = x.shape[0]
    assert B == 8

    nc = _build_nc(*_prep_weights(inputs))

    in_maps = []
    for i in range(B):
        xi = x[i].reshape(C, N)
        yi = y[i].reshape(C, N)
        zi = z[i].reshape(C, N)
        in_maps.append({
            "xy": _bf(np.concatenate([xi, yi], axis=0)),
            "z": _bf(zi),
            "xc": _canvas(x[i], NP_E4),
            "yc": _canvas(y[i], NP_E4),
            "zc": _canvas(z[i] * Z8_SCALE, NP_E5),
        })
    res = run_bass_kernel_spmd(nc, in_maps, list(range(8)))
    out = np.stack(
        [np.asarray(res.results[i]["out"]).astype(np.float32).reshape(DIM, H, W)
         for i in range(B)]
    )
    return out
